# revision 1
# baseline (speedup 1.0000x reference)
"""Distributed HGT message-passing kernel for 8 Trainium2 NeuronCores.

Sharding (dst-sharded graph parallel, per the hint):
  - Destination nodes of both types are partitioned row-wise across 8 cores
    (6250/type/core); each relation's edges are routed to the owner of their
    destination node (host-side index preprocessing).
  - Each core projects K/V only for the unique *boundary source nodes* its
    edges reference (staged per-core), building compact per-relation tables
    kv_r = [k_raw || v @ mrel_r].  The relation key-transform is folded into
    the query side (q~ = q @ arel^T * prel/sqrt(D)), so k stays raw.
  - Edge phase: edges grouped into 128-dst blocks x 128-edge tiles; host-built
    one-hot dst masks M drive q~ row expansion and the segment-softmax
    numerator/denominator as PE matmuls.  Softmax skips max-subtraction
    (logits are O(1)); exp runs in fp32.
  - Per-relation [H,D,D] weights are replicated and fused into the projection
    weights on device.
"""

import math
import sys
from contextlib import ExitStack

import numpy as np
import ml_dtypes

sys.path.insert(0, "/opt/trn_rl_repo")

import concourse.bass as bass  # noqa: E402,F401
import concourse.mybir as mybir  # noqa: E402
import concourse.tile as tile  # noqa: E402
from concourse import bacc  # noqa: E402
from concourse.bass_utils import run_bass_kernel_spmd  # noqa: E402
from concourse.masks import make_identity  # noqa: E402

BF16 = ml_dtypes.bfloat16
N, E, C, H, D = 50000, 200000, 512, 8, 64
NCORES = 8
NSH = N // NCORES          # 6250 dst nodes per type per core
P = 128
NBLK = (NSH + P - 1) // P  # 49 dst blocks
NSHP = NBLK * P            # 6272 padded
TB = 5                     # edge tiles per dst block (640 edge slots)
NTILES = NBLK * TB         # 245 tiles per relation per core
UCAP = 19968               # compact table rows (mult of 512, > max unique)
RELS = [("r1", "B", "A"), ("r2", "A", "B"), ("r3", "A", "A")]
TYPE_RELS = [("B", ["r1"]), ("A", ["r2", "r3"])]

f32 = mybir.dt.float32
bf = mybir.dt.bfloat16
i16 = mybir.dt.int16
AF = mybir.ActivationFunctionType
OP = mybir.AluOpType


# ---------------------------------------------------------------------------
# Host-side preprocessing (index routing + layout staging only)
# ---------------------------------------------------------------------------

def _prep_core(core, inp):
    m = {}
    lo = core * NSH
    for t in ("A", "B"):
        x = inp[f"x_{t}"]
        xq = np.zeros((C, NSHP), BF16)
        xq[:, :NSH] = x[lo:lo + NSH].T.astype(BF16)
        m[f"x{t}T_q"] = xq
        xo = np.zeros((NSHP, C), np.float32)
        xo[:NSH] = x[lo:lo + NSH]
        m[f"x{t}_own"] = xo

    for r, T, S in RELS:
        ei = inp[f"ei_{r}"]
        src, dst = ei[0], ei[1]
        sel = (dst >= lo) & (dst < lo + NSH)
        src, dst = src[sel], dst[sel] - lo
        usrc, pos = np.unique(src, return_inverse=True)
        assert len(usrc) <= UCAP, len(usrc)
        blk = dst // P
        cnt = np.bincount(blk, minlength=NBLK)
        assert cnt.max() <= TB * P, cnt.max()
        order = np.argsort(blk, kind="stable")
        pos, dloc = pos[order].astype(np.int64), (dst[order] % P)

        idx_flat = np.zeros(NTILES * P, np.int16)
        dl_flat = -np.ones(NTILES * P, np.int64)
        off = 0
        for b in range(NBLK):
            nb_e = cnt[b]
            base = b * TB * P
            idx_flat[base:base + nb_e] = pos[off:off + nb_e]
            dl_flat[base:base + nb_e] = dloc[off:off + nb_e]
            off += nb_e

        lay = idx_flat.reshape(-1, 16).T          # idx i -> (part i%16, col i//16)
        m[f"idx_{r}"] = np.tile(lay, (8, 1)).copy()

        Mm = np.zeros((P, NTILES * P), BF16)      # one-hot dst masks [d, (t e)]
        cols = np.nonzero(dl_flat >= 0)[0]
        Mm[dl_flat[cols], cols] = 1.0
        m[f"M_{r}"] = Mm
        Mt = np.zeros((P, NTILES * P), BF16)      # transposed masks [e, (t d)]
        Mt[cols % P, (cols // P) * P + dl_flat[cols]] = 1.0
        m[f"MT_{r}"] = Mt

        xs = np.zeros((C, UCAP), BF16)
        xs[:, :len(usrc)] = inp[f"x_{S}"][usrc].T.astype(BF16)
        m[f"xsT_{r}"] = xs
    return m


def _prep_shared(inp):
    m = {}
    sD = 1.0 / math.sqrt(D)
    for t in ("A", "B"):
        m[f"kW_{t}"] = inp[f"kW_{t}"].reshape(4, P, C).astype(BF16)
        m[f"vWT_{t}"] = np.ascontiguousarray(inp[f"vW_{t}"].T).reshape(8, D, C).astype(BF16)
        m[f"qWT_{t}"] = np.ascontiguousarray(inp[f"qW_{t}"].T).reshape(8, D, C).astype(BF16)
        m[f"oW_{t}"] = inp[f"oW_{t}"].reshape(4, P, C).astype(BF16)
        m[f"skip_{t}"] = np.full((P, 1), float(inp[f"skip_{t}"]), np.float32)
    m["linW"] = inp["linW"].reshape(4, P, 128).astype(BF16)
    for r, _, _ in RELS:
        m[f"mrel_{r}"] = np.ascontiguousarray(
            inp[f"mrel_{r}"].transpose(1, 0, 2)).reshape(D, C).astype(BF16)
        at = inp[f"arel_{r}"] * (inp[f"prel_{r}"] * sD)[:, None, None]
        m[f"arelT_{r}"] = np.ascontiguousarray(
            at.transpose(2, 0, 1)).reshape(D, C).astype(BF16)
    for nm in ("kb_A", "kb_B", "ob_A", "ob_B", "linb"):
        m[nm] = np.tile(np.asarray(inp[nm], np.float32)[None, :], (P, 1))
    for t in ("A", "B"):
        for pfx in ("q", "v"):
            m[f"{pfx}b_{t}"] = np.ascontiguousarray(
                np.asarray(inp[f"{pfx}b_{t}"], np.float32).reshape(8, D).T)
    return m


# ---------------------------------------------------------------------------
# Device program
# ---------------------------------------------------------------------------

def _build(bz):
    nc = bacc.Bacc("TRN2", target_bir_lowering=False, debug=False,
                   enable_asserts=False, num_devices=NCORES)
    inp = {}

    def di(name, shape, dt):
        inp[name] = nc.dram_tensor(name, shape, dt, kind="ExternalInput").ap()

    for t in ("A", "B"):
        di(f"x{t}T_q", [C, NSHP], bf)
        di(f"x{t}_own", [NSHP, C], f32)
        di(f"kW_{t}", [4, P, C], bf)
        di(f"vWT_{t}", [8, D, C], bf)
        di(f"qWT_{t}", [8, D, C], bf)
        di(f"oW_{t}", [4, P, C], bf)
        di(f"skip_{t}", [P, 1], f32)
        di(f"kb_{t}", [P, C], f32)
        di(f"ob_{t}", [P, C], f32)
        di(f"qb_{t}", [D, 8], f32)
        di(f"vb_{t}", [D, 8], f32)
    di("linW", [4, P, 128], bf)
    di("linb", [P, 128], f32)
    for r, _, _ in RELS:
        di(f"mrel_{r}", [D, C], bf)
        di(f"arelT_{r}", [D, C], bf)
        di(f"idx_{r}", [P, NTILES * 8], i16)
        di(f"M_{r}", [P, NTILES * P], bf)
        di(f"MT_{r}", [P, NTILES * P], bf)
        di(f"xsT_{r}", [C, UCAP], bf)
    out = nc.dram_tensor("out", [2 * NSHP, 128], f32, kind="ExternalOutput").ap()

    with tile.TileContext(nc) as tc:
        with ExitStack() as es:
            _program(es, tc, inp, out, bz)
    nc.compile()
    return nc


def _program(es, tc, inp, out, bz):
    nc = tc.nc
    wp = es.enter_context(tc.tile_pool(name="w", bufs=1))
    dp = es.enter_context(tc.tile_pool(name="d", bufs=1, space="DRAM"))
    sp = es.enter_context(tc.tile_pool(name="s", bufs=2))
    ep = es.enter_context(tc.tile_pool(name="e", bufs=2))
    gp = es.enter_context(tc.tile_pool(name="g", bufs=2))
    pp = es.enter_context(tc.tile_pool(name="p", bufs=3, space="PSUM"))
    agp = es.enter_context(tc.tile_pool(name="a", bufs=2, space="PSUM"))
    dnp = es.enter_context(tc.tile_pool(name="n", bufs=2, space="PSUM"))

    ident = wp.tile([P, P], bf, tag="ident", name="ident")
    make_identity(nc, ident[:])

    def load_w(name, shape=(P, 4, C), dt=bf, rearr="c p o -> p c o"):
        t = wp.tile(list(shape), dt, tag=name)
        nc.sync.dma_start(t[:], inp[name].rearrange(rearr))
        return t

    kW = {t: load_w(f"kW_{t}") for t in ("A", "B")}
    vWT = {t: load_w(f"vWT_{t}", (D, 8, C), bf, "h p o -> p h o") for t in ("A", "B")}
    qWT = {t: load_w(f"qWT_{t}", (D, 8, C), bf, "h p o -> p h o") for t in ("A", "B")}
    oW = {t: load_w(f"oW_{t}") for t in ("A", "B")}
    linW = load_w("linW", (P, 4, 128))
    mrel, arelT = {}, {}
    for r, _, _ in RELS:
        mrel[r] = wp.tile([D, C], bf, tag=f"mrel{r}", name=f"mrel{r}")
        nc.sync.dma_start(mrel[r][:], inp[f"mrel_{r}"])
        arelT[r] = wp.tile([D, C], bf, tag=f"arelT{r}", name=f"arelT{r}")
        nc.sync.dma_start(arelT[r][:], inp[f"arelT_{r}"])

    kb, ob, qb, vb, gate, gate1m = {}, {}, {}, {}, {}, {}
    for t in ("A", "B"):
        for pfx, dd, shape in (("kb", kb, [P, C]), ("ob", ob, [P, C]),
                               ("qb", qb, [D, 8]), ("vb", vb, [D, 8])):
            if not bz[f"{pfx}_{t}"]:
                tt_ = wp.tile(shape, f32, tag=f"{pfx}{t}", name=f"{pfx}{t}")
                nc.sync.dma_start(tt_[:], inp[f"{pfx}_{t}"])
                dd[t] = tt_
        sk = wp.tile([P, 1], f32, tag=f"sk{t}", name=f"sk{t}")
        nc.sync.dma_start(sk[:], inp[f"skip_{t}"])
        g_ = wp.tile([P, 1], f32, tag=f"g{t}", name=f"g{t}")
        nc.scalar.activation(g_[:], sk[:], AF.Sigmoid)
        gate[t] = g_
        g1 = wp.tile([P, 1], f32, tag=f"g1{t}", name=f"g1{t}")
        nc.vector.tensor_scalar(g1[:], g_[:], -1.0, 1.0, OP.mult, OP.add)
        gate1m[t] = g1
    linb = None
    if not bz["linb"]:
        linb = wp.tile([P, 128], f32, tag="linb", name="linb")
        nc.sync.dma_start(linb[:], inp["linb"])

    idx_t, qt_dram, kv_dram = {}, {}, {}
    for r, _, _ in RELS:
        it_ = wp.tile([P, NTILES * 8], i16, tag=f"idx{r}", name=f"idx{r}")
        nc.sync.dma_start(it_[:], inp[f"idx_{r}"])
        idx_t[r] = it_
        qt_dram[r] = dp.tile([NSHP, C], bf, tag=f"qtd{r}", name=f"qtd{r}")
        kv_dram[r] = dp.tile([UCAP, 2 * C], bf, tag=f"kvd{r}", name=f"kvd{r}")

    # ---- stage 0: fuse relation transforms into projection weights ----
    Wv, Wq = {}, {}
    for r, T, S in RELS:
        for nm, Wd, WT, rel_w in (("v", Wv, vWT[S], mrel[r]),
                                  ("q", Wq, qWT[T], arelT[r])):
            Wt = wp.tile([P, 4, C], bf, tag=f"W{nm}{r}", name=f"W{nm}{r}")
            for cc in range(4):
                ps = pp.tile([P, C], f32, tag="ps", name="ps")
                for h in range(H):
                    nc.tensor.matmul(
                        ps[:, h * D:(h + 1) * D],
                        WT[:, h, cc * P:(cc + 1) * P],
                        rel_w[:, h * D:(h + 1) * D],
                        start=True, stop=True)
                nc.scalar.copy(Wt[:, cc, :], ps[:])
            Wd[r] = Wt

    qbr, vbr = {}, {}
    for r, T, S in RELS:
        for dd, src_b, rel_w in ((vbr, vb.get(S), mrel[r]),
                                 (qbr, qb.get(T), arelT[r])):
            if src_b is None:
                continue
            ps = pp.tile([P, C], f32, tag="ps", name="ps")
            for h in range(H):
                nc.tensor.matmul(ps[:1, h * D:(h + 1) * D],
                                 src_b[:, h:h + 1],
                                 rel_w[:, h * D:(h + 1) * D],
                                 start=True, stop=True)
            sb_ = wp.tile([1, C], f32, tag=f"bs{r}{len(dd)}", name=f"bs{r}{len(dd)}")
            nc.vector.tensor_copy(sb_[:], ps[:1, :])
            rep = wp.tile([P, C], f32, tag=f"br{r}{len(dd)}", name=f"br{r}{len(dd)}")
            nc.gpsimd.partition_broadcast(rep[:], sb_[:])
            dd[r] = rep

    # ---- stage 1: q~ tables (own dst shard) ----
    for r, T, S in RELS:
        xin = inp[f"x{T}T_q"]
        nt_list = [4] * (NSHP // 512) + ([1] if (NSHP % 512) else [])
        i0 = 0
        for ntile in nt_list:
            w = ntile * P
            xs = sp.tile([P, 4, 512], bf, tag="xq", name="xq")
            nc.sync.dma_start(xs[:, :, :w],
                              xin[:, i0:i0 + w].rearrange("(cc p) i -> p cc i", p=P))
            qs = sp.tile([P, 4, C], bf, tag="qs", name="qs")
            for t in range(ntile):
                ps = pp.tile([P, C], f32, tag="ps", name="ps")
                for cc in range(4):
                    nc.tensor.matmul(ps[:], xs[:, cc, t * P:(t + 1) * P],
                                     Wq[r][:, cc, :], start=(cc == 0), stop=(cc == 3))
                if r in qbr:
                    nc.vector.tensor_tensor(qs[:, t, :], ps[:], qbr[r][:], OP.add)
                else:
                    nc.scalar.copy(qs[:, t, :], ps[:])
            nc.sync.dma_start(
                qt_dram[r][:][i0:i0 + w, :].rearrange("(g p) c -> p g c", p=P),
                qs[:, :ntile, :])
            i0 += w

    # ---- stage 2: kv tables (compact unique sources) ----
    for r, T, S in RELS:
        xin = inp[f"xsT_{r}"]
        for ib in range(UCAP // 512):
            i0 = ib * 512
            xs = sp.tile([P, 4, 512], bf, tag="xs2", name="xs2")
            nc.sync.dma_start(
                xs[:], xin[:, i0:i0 + 512].rearrange("(cc p) i -> p cc i", p=P))
            kvs = sp.tile([P, 4, 2 * C], bf, tag="kvs", name="kvs")
            for t in range(4):
                psk = pp.tile([P, C], f32, tag="ps", name="ps")
                for cc in range(4):
                    nc.tensor.matmul(psk[:], xs[:, cc, t * P:(t + 1) * P],
                                     kW[S][:, cc, :], start=(cc == 0), stop=(cc == 3))
                if S in kb:
                    nc.vector.tensor_tensor(kvs[:, t, 0:C], psk[:], kb[S][:], OP.add)
                else:
                    nc.scalar.copy(kvs[:, t, 0:C], psk[:])
                psv = pp.tile([P, C], f32, tag="ps", name="ps")
                for cc in range(4):
                    nc.tensor.matmul(psv[:], xs[:, cc, t * P:(t + 1) * P],
                                     Wv[r][:, cc, :], start=(cc == 0), stop=(cc == 3))
                if r in vbr:
                    nc.vector.tensor_tensor(kvs[:, t, C:], psv[:], vbr[r][:], OP.add)
                else:
                    nc.scalar.copy(kvs[:, t, C:], psv[:])
            nc.sync.dma_start(
                kv_dram[r][:][i0:i0 + 512, :].rearrange("(g p) c -> p g c", p=P),
                kvs[:])

    # ---- stage 3: edge phase + output, per dst block ----
    for T, rels in TYPE_RELS:
        toff = 0 if T == "A" else NSHP
        for blk in range(NBLK):
            xo = ep.tile([P, C], f32, tag="xo", name="xo")
            nc.sync.dma_start(xo[:], inp[f"x{T}_own"][blk * P:(blk + 1) * P, :])
            norms = []
            for r in rels:
                kv = gp.tile([P, TB, 2 * C], bf, tag="kv", name="kv")
                nc.gpsimd.dma_gather(
                    kv[:], kv_dram[r][:],
                    idx_t[r][:, blk * TB * 8:(blk + 1) * TB * 8],
                    TB * P, TB * P, 2 * C)
                mt = gp.tile([P, TB, P], bf, tag="mt", name="mt")
                nc.sync.dma_start(
                    mt[:], inp[f"M_{r}"][:, blk * TB * P:(blk + 1) * TB * P]
                    .rearrange("p (t e) -> p t e", e=P))
                mtT = gp.tile([P, TB, P], bf, tag="mtT", name="mtT")
                nc.sync.dma_start(
                    mtT[:], inp[f"MT_{r}"][:, blk * TB * P:(blk + 1) * TB * P]
                    .rearrange("p (t e) -> p t e", e=P))
                qt = ep.tile([P, C], bf, tag="qt", name="qt")
                nc.sync.dma_start(qt[:], qt_dram[r][:][blk * P:(blk + 1) * P, :])
                agg = agp.tile([P, C], f32, tag="agg", name="agg")
                den = dnp.tile([P, H], f32, tag="den", name="den")
                for t in range(TB):
                    qe = pp.tile([P, C], f32, tag="ps", name="ps")
                    nc.tensor.matmul(qe[:], mt[:, t, :], qt[:], start=True, stop=True)
                    qeb = ep.tile([P, C], bf, tag="qeb", name="qeb")
                    nc.scalar.copy(qeb[:], qe[:])
                    prod = ep.tile([P, C], bf, tag="prod", name="prod")
                    nc.vector.tensor_tensor(prod[:], kv[:, t, 0:C], qeb[:], OP.mult)
                    L = ep.tile([P, H], f32, tag="L", name="L")
                    nc.vector.tensor_reduce(
                        L[:], prod[:].rearrange("p (h c) -> p h c", h=H),
                        axis=mybir.AxisListType.X, op=OP.add)
                    aT = ep.tile([P, H], bf, tag="aT", name="aT")
                    nc.scalar.activation(aT[:], L[:], AF.Exp)
                    va = ep.tile([P, C], bf, tag="va", name="va")
                    nc.vector.tensor_tensor(
                        va[:].rearrange("p (h c) -> p h c", h=H),
                        kv[:, t, C:].rearrange("p (h c) -> p h c", h=H),
                        aT[:].rearrange("p (h o) -> p h o", o=1).to_broadcast([P, H, D]),
                        OP.mult)
                    nc.tensor.matmul(agg[:], mtT[:, t, :], va[:],
                                     start=(t == 0), stop=(t == TB - 1))
                    nc.tensor.matmul(den[:], mtT[:, t, :], aT[:],
                                     start=(t == 0), stop=(t == TB - 1))
                dn = ep.tile([P, H], f32, tag="dn", name="dn")
                nc.vector.tensor_scalar_add(dn[:], den[:], 1e-16)
                rec = ep.tile([P, H], f32, tag="rec", name="rec")
                nc.vector.reciprocal(rec[:], dn[:])
                nrm = ep.tile([P, C], f32 if len(rels) > 1 else bf,
                              tag=f"nrm{len(norms)}", name=f"nrm{len(norms)}")
                nc.vector.tensor_tensor(
                    nrm[:].rearrange("p (h c) -> p h c", h=H),
                    agg[:].rearrange("p (h c) -> p h c", h=H),
                    rec[:].rearrange("p (h o) -> p h o", o=1).to_broadcast([P, H, D]),
                    OP.mult)
                norms.append(nrm)
            if len(norms) > 1:
                gsum = ep.tile([P, C], bf, tag="gsum", name="gsum")
                nc.vector.tensor_tensor(gsum[:], norms[0][:], norms[1][:], OP.add)
            else:
                gsum = norms[0]
            gel = ep.tile([P, C], bf, tag="gel", name="gel")
            nc.scalar.activation(gel[:], gsum[:], AF.Gelu)
            gT = ep.tile([P, 4, P], bf, tag="gT", name="gT")
            for cc in range(4):
                tp = pp.tile([P, P], bf, tag="ps", name="ps")
                nc.tensor.transpose(tp[:], gel[:, cc * P:(cc + 1) * P], ident[:])
                nc.scalar.copy(gT[:, cc, :], tp[:])
            o_ps = pp.tile([P, C], f32, tag="ps", name="ps")
            for cc in range(4):
                nc.tensor.matmul(o_ps[:], gT[:, cc, :], oW[T][:, cc, :],
                                 start=(cc == 0), stop=(cc == 3))
            if T in ob:
                nc.vector.tensor_tensor(o_ps[:], o_ps[:], ob[T][:], OP.add)
            xg = ep.tile([P, C], f32, tag="xg", name="xg")
            nc.scalar.activation(xg[:], xo[:], AF.Copy, scale=gate1m[T][:])
            hb = ep.tile([P, C], bf, tag="hb", name="hb")
            nc.vector.scalar_tensor_tensor(hb[:], o_ps[:], gate[T][:], xg[:],
                                           OP.mult, OP.add)
            hT = ep.tile([P, 4, P], bf, tag="hT", name="hT")
            for cc in range(4):
                tp = pp.tile([P, P], bf, tag="ps", name="ps")
                nc.tensor.transpose(tp[:], hb[:, cc * P:(cc + 1) * P], ident[:])
                nc.scalar.copy(hT[:, cc, :], tp[:])
            fin = pp.tile([P, 128], f32, tag="ps", name="ps")
            for cc in range(4):
                nc.tensor.matmul(fin[:], hT[:, cc, :], linW[:, cc, :],
                                 start=(cc == 0), stop=(cc == 3))
            fo = ep.tile([P, 128], f32, tag="fo", name="fo")
            if linb is not None:
                nc.vector.tensor_tensor(fo[:], fin[:], linb[:], OP.add)
            else:
                nc.scalar.copy(fo[:], fin[:])
            nc.sync.dma_start(out[toff + blk * P: toff + (blk + 1) * P, :], fo[:])


# ---------------------------------------------------------------------------
# Entry point
# ---------------------------------------------------------------------------

_CACHE = {}


def kernel(**inputs):
    inp = {k: np.asarray(v) for k, v in inputs.items()}
    shared = _prep_shared(inp)
    bz = {k: not np.any(shared[k]) for k in
          ("kb_A", "kb_B", "ob_A", "ob_B", "linb",
           "qb_A", "qb_B", "vb_A", "vb_B")}
    key = tuple(sorted(bz.items()))
    if key not in _CACHE:
        _CACHE[key] = _build(bz)
    nc = _CACHE[key]

    in_maps = []
    for core in range(NCORES):
        m = dict(shared)
        m.update(_prep_core(core, inp))
        in_maps.append(m)

    import time as _time
    _t0 = _time.time()
    res = run_bass_kernel_spmd(nc, in_maps, core_ids=list(range(NCORES)))
    kernel.last_run_s = _time.time() - _t0
    kernel.last_results = res

    full = np.zeros((2 * N, 128), np.float32)
    for core in range(NCORES):
        o = res.results[core]["out"]
        full[core * NSH:(core + 1) * NSH] = o[:NSH]
        full[N + core * NSH:N + (core + 1) * NSH] = o[NSHP:NSHP + NSH]
    return full



# revision 2
# speedup vs baseline: 4.4328x; 4.4328x over previous
"""Distributed HGT message-passing kernel for 8 Trainium2 NeuronCores.

Wire-optimized dst-sharded design (the host->device transfer dominates
end-to-end time, so inputs are kept minimal and everything derivable is
built on device):
  - Each core ships only its own dst shard of x (bf16), the replicated
    weights (bf16), and compact int16 edge-routing indices.
  - An on-device AllGather assembles the full node-feature table; each
    core then gathers the unique boundary source features its edges
    reference (transposed dma_gather, per-owner-bucket bases so indices
    fit int16) and builds compact kv_r = [k_raw || v @ mrel_r] tables.
  - One-hot dst masks for the edge phase are built on device from
    shipped dst-local ids (iota compare + PE transpose), not shipped.
  - The relation key-transform is folded into the query side
    (q~ = q @ arel^T * prel/sqrt(D)); softmax skips max-subtraction
    (logits are O(1)); exp runs in fp32.  Skip path and final output
    are bf16 (host upcasts).
"""

import math
import sys
from contextlib import ExitStack

import numpy as np
import ml_dtypes

sys.path.insert(0, "/opt/trn_rl_repo")

import concourse.bass as bass  # noqa: E402,F401
import concourse.mybir as mybir  # noqa: E402
import concourse.tile as tile  # noqa: E402
from concourse import bacc  # noqa: E402
from concourse.bass_utils import run_bass_kernel_spmd  # noqa: E402
from concourse.masks import make_identity  # noqa: E402

BF16 = ml_dtypes.bfloat16
N, E, C, H, D = 50000, 200000, 512, 8, 64
NCORES = 8
NSH = N // NCORES          # 6250 dst nodes per type per core
P = 128
NBLK = (NSH + P - 1) // P  # 49 dst blocks
NSHP = NBLK * P            # 6272 padded
XROWS = 2 * NSHP           # 12544 rows per core in the x shard (A then B)
XFULL = NCORES * XROWS     # 100352 rows after AllGather
TB = 5                     # edge tiles per dst block (640 edge slots)
NTILES = NBLK * TB         # 245 tiles per relation per core
UCB = 3072                 # compact-table rows per owner bucket (6 x 512)
UCHUNK = UCB // 512        # gather chunks per bucket
UCAP = NCORES * UCB        # 24576 compact rows per relation (< int16 max)
RELS = [("r1", "B", "A"), ("r2", "A", "B"), ("r3", "A", "A")]
TYPE_RELS = [("B", ["r1"]), ("A", ["r2", "r3"])]

f32 = mybir.dt.float32
bf = mybir.dt.bfloat16
i16 = mybir.dt.int16
AF = mybir.ActivationFunctionType
OP = mybir.AluOpType


# ---------------------------------------------------------------------------
# Host-side preprocessing (index routing + layout staging only)
# ---------------------------------------------------------------------------

def _wrap16(flat):
    """[n] -> [128, n//16] gather-index layout (16-partition wrap, 8x rep)."""
    lay = flat.reshape(-1, 16).T
    return np.tile(lay, (8, 1)).copy()


def _prep_core(core, inp):
    m = {}
    lo = core * NSH
    xsh = np.zeros((XROWS, C), BF16)
    xsh[:NSH] = inp["x_A"][lo:lo + NSH].astype(BF16)
    xsh[NSHP:NSHP + NSH] = inp["x_B"][lo:lo + NSH].astype(BF16)
    m["xsh"] = xsh

    for r, T, S in RELS:
        ei = inp[f"ei_{r}"]
        src, dst = ei[0], ei[1]
        sel = (dst >= lo) & (dst < lo + NSH)
        src, dst = src[sel], dst[sel] - lo

        usrc, pos = np.unique(src, return_inverse=True)
        owner = usrc // NSH
        bcnt = np.bincount(owner, minlength=NCORES)
        assert bcnt.max() <= UCB, bcnt.max()
        # compact row = owner*UCB + rank within owner bucket
        boff = np.zeros(NCORES + 1, np.int64)
        boff[1:] = np.cumsum(bcnt)
        crow = np.empty(len(usrc), np.int64)
        crow[:] = owner * UCB + (np.arange(len(usrc)) - boff[owner])
        # local row within the owner's per-type x region
        ulocal = np.zeros(UCAP, np.int64)
        ulocal[crow] = usrc % NSH
        m[f"uidx_{r}"] = _wrap16(ulocal.astype(np.int16))

        blk = dst // P
        cnt = np.bincount(blk, minlength=NBLK)
        assert cnt.max() <= TB * P, cnt.max()
        order = np.argsort(blk, kind="stable")
        epos = crow[pos][order]
        dloc = (dst[order] % P)

        eidx_flat = np.zeros(NTILES * P, np.int16)
        dl_flat = np.full(NTILES * P, 200.0, np.float64)  # 200 => empty slot
        off = 0
        for b in range(NBLK):
            nb_e = cnt[b]
            base = b * TB * P
            eidx_flat[base:base + nb_e] = epos[off:off + nb_e]
            dl_flat[base:base + nb_e] = dloc[off:off + nb_e]
            off += nb_e
        m[f"eidx_{r}"] = _wrap16(eidx_flat)
        # dst-local per edge slot: [128 slot-in-tile, NTILES]
        m[f"dl_{r}"] = np.ascontiguousarray(
            dl_flat.reshape(NTILES, P).T).astype(BF16)
    return m


def _prep_shared(inp):
    m = {}
    sD = 1.0 / math.sqrt(D)
    for t in ("A", "B"):
        m[f"kW_{t}"] = inp[f"kW_{t}"].reshape(4, P, C).astype(BF16)
        m[f"vWT_{t}"] = np.ascontiguousarray(inp[f"vW_{t}"].T).reshape(8, D, C).astype(BF16)
        m[f"qWT_{t}"] = np.ascontiguousarray(inp[f"qW_{t}"].T).reshape(8, D, C).astype(BF16)
        m[f"oW_{t}"] = inp[f"oW_{t}"].reshape(4, P, C).astype(BF16)
        m[f"skip_{t}"] = np.full((1, 1), float(inp[f"skip_{t}"]), np.float32)
    m["linW"] = inp["linW"].reshape(4, P, 128).astype(BF16)
    for r, _, _ in RELS:
        m[f"mrel_{r}"] = np.ascontiguousarray(
            inp[f"mrel_{r}"].transpose(1, 0, 2)).reshape(D, C).astype(BF16)
        at = inp[f"arel_{r}"] * (inp[f"prel_{r}"] * sD)[:, None, None]
        m[f"arelT_{r}"] = np.ascontiguousarray(
            at.transpose(2, 0, 1)).reshape(D, C).astype(BF16)
    for nm in ("kb_A", "kb_B", "ob_A", "ob_B"):
        m[nm] = np.asarray(inp[nm], np.float32).reshape(1, C)
    m["linb"] = np.asarray(inp["linb"], np.float32).reshape(1, 128)
    for t in ("A", "B"):
        for pfx in ("q", "v"):
            m[f"{pfx}b_{t}"] = np.ascontiguousarray(
                np.asarray(inp[f"{pfx}b_{t}"], np.float32).reshape(8, D).T)
    m["iota"] = np.tile(np.arange(P, dtype=np.float32).astype(BF16), (P, 1))
    return m


# ---------------------------------------------------------------------------
# Device program
# ---------------------------------------------------------------------------

def _build(bz):
    nc = bacc.Bacc("TRN2", target_bir_lowering=False, debug=False,
                   enable_asserts=False, num_devices=NCORES)
    inp = {}

    def di(name, shape, dt):
        inp[name] = nc.dram_tensor(name, shape, dt, kind="ExternalInput").ap()

    di("xsh", [XROWS, C], bf)
    di("iota", [P, P], bf)
    for t in ("A", "B"):
        di(f"kW_{t}", [4, P, C], bf)
        di(f"vWT_{t}", [8, D, C], bf)
        di(f"qWT_{t}", [8, D, C], bf)
        di(f"oW_{t}", [4, P, C], bf)
        di(f"skip_{t}", [1, 1], f32)
        if not bz[f"kb_{t}"]:
            di(f"kb_{t}", [1, C], f32)
        if not bz[f"ob_{t}"]:
            di(f"ob_{t}", [1, C], f32)
        if not bz[f"qb_{t}"]:
            di(f"qb_{t}", [D, 8], f32)
        if not bz[f"vb_{t}"]:
            di(f"vb_{t}", [D, 8], f32)
    di("linW", [4, P, 128], bf)
    if not bz["linb"]:
        di("linb", [1, 128], f32)
    for r, _, _ in RELS:
        di(f"mrel_{r}", [D, C], bf)
        di(f"arelT_{r}", [D, C], bf)
        di(f"uidx_{r}", [P, UCAP // 16], i16)
        di(f"eidx_{r}", [P, NTILES * 8], i16)
        di(f"dl_{r}", [P, NTILES], bf)
    out = nc.dram_tensor("out", [XROWS, 128], bf, kind="ExternalOutput").ap()

    with tile.TileContext(nc) as tc:
        with ExitStack() as es:
            _program(es, tc, inp, out, bz)
    nc.compile()
    return nc


def _program(es, tc, inp, out, bz):
    nc = tc.nc
    wp = es.enter_context(tc.tile_pool(name="w", bufs=1))
    dp = es.enter_context(tc.tile_pool(name="d", bufs=1, space="DRAM"))
    sp = es.enter_context(tc.tile_pool(name="s", bufs=2))
    ep = es.enter_context(tc.tile_pool(name="e", bufs=2))
    gp = es.enter_context(tc.tile_pool(name="g", bufs=2))
    pp = es.enter_context(tc.tile_pool(name="p", bufs=3, space="PSUM"))
    agp = es.enter_context(tc.tile_pool(name="a", bufs=2, space="PSUM"))
    dnp = es.enter_context(tc.tile_pool(name="n", bufs=2, space="PSUM"))

    ident = wp.tile([P, P], bf, tag="ident", name="ident")
    make_identity(nc, ident[:])
    iota = wp.tile([P, P], bf, tag="iota", name="iota")
    nc.sync.dma_start(iota[:], inp["iota"])

    # ---- all-gather the x shards into the full node table ----
    xint = dp.tile([XROWS, C], bf, tag="xint", name="xint")
    xfull = dp.tile([XFULL, C], bf, tag="xfull", name="xfull",
                    addr_space="Shared")
    nc.gpsimd.dma_start(xint[:], inp["xsh"])
    nc.gpsimd.collective_compute(
        "AllGather", OP.bypass,
        replica_groups=[list(range(NCORES))],
        ins=[xint.opt()],
        outs=[xfull.opt()],
    )

    def load_w(name, shape=(P, 4, C), dt=bf, rearr="c p o -> p c o"):
        t = wp.tile(list(shape), dt, tag=name)
        nc.sync.dma_start(t[:], inp[name].rearrange(rearr))
        return t

    kW = {t: load_w(f"kW_{t}") for t in ("A", "B")}
    vWT = {t: load_w(f"vWT_{t}", (D, 8, C), bf, "h p o -> p h o") for t in ("A", "B")}
    qWT = {t: load_w(f"qWT_{t}", (D, 8, C), bf, "h p o -> p h o") for t in ("A", "B")}
    oW = {t: load_w(f"oW_{t}") for t in ("A", "B")}
    linW = load_w("linW", (P, 4, 128))
    mrel, arelT = {}, {}
    for r, _, _ in RELS:
        mrel[r] = wp.tile([D, C], bf, tag=f"mrel{r}", name=f"mrel{r}")
        nc.sync.dma_start(mrel[r][:], inp[f"mrel_{r}"])
        arelT[r] = wp.tile([D, C], bf, tag=f"arelT{r}", name=f"arelT{r}")
        nc.sync.dma_start(arelT[r][:], inp[f"arelT_{r}"])

    kb, ob, qb, vb, gate, gate1m = {}, {}, {}, {}, {}, {}
    for t in ("A", "B"):
        for pfx, dd in (("kb", kb), ("ob", ob)):
            if not bz[f"{pfx}_{t}"]:
                row = wp.tile([1, C], f32, tag=f"{pfx}{t}r", name=f"{pfx}{t}r")
                nc.sync.dma_start(row[:], inp[f"{pfx}_{t}"])
                tt_ = wp.tile([P, C], f32, tag=f"{pfx}{t}", name=f"{pfx}{t}")
                nc.gpsimd.partition_broadcast(tt_[:], row[:])
                dd[t] = tt_
        for pfx, dd in (("qb", qb), ("vb", vb)):
            if not bz[f"{pfx}_{t}"]:
                tt_ = wp.tile([D, 8], f32, tag=f"{pfx}{t}", name=f"{pfx}{t}")
                nc.sync.dma_start(tt_[:], inp[f"{pfx}_{t}"])
                dd[t] = tt_
        sk = wp.tile([1, 1], f32, tag=f"sk{t}", name=f"sk{t}")
        nc.sync.dma_start(sk[:], inp[f"skip_{t}"])
        g1_ = wp.tile([1, 1], f32, tag=f"g1r{t}", name=f"g1r{t}")
        nc.scalar.activation(g1_[:], sk[:], AF.Sigmoid)
        g_ = wp.tile([P, 1], f32, tag=f"g{t}", name=f"g{t}")
        nc.gpsimd.partition_broadcast(g_[:], g1_[:])
        gate[t] = g_
        g1 = wp.tile([P, 1], f32, tag=f"g1{t}", name=f"g1{t}")
        nc.vector.tensor_scalar(g1[:], g_[:], -1.0, 1.0, OP.mult, OP.add)
        gate1m[t] = g1
    linb = None
    if not bz["linb"]:
        lrow = wp.tile([1, 128], f32, tag="linbr", name="linbr")
        nc.sync.dma_start(lrow[:], inp["linb"])
        linb = wp.tile([P, 128], f32, tag="linb", name="linb")
        nc.gpsimd.partition_broadcast(linb[:], lrow[:])

    uidx_t, eidx_t, dl_t, qt_dram, kv_dram = {}, {}, {}, {}, {}
    for r, _, _ in RELS:
        ut_ = wp.tile([P, UCAP // 16], i16, tag=f"uidx{r}", name=f"uidx{r}")
        nc.sync.dma_start(ut_[:], inp[f"uidx_{r}"])
        uidx_t[r] = ut_
        it_ = wp.tile([P, NTILES * 8], i16, tag=f"eidx{r}", name=f"eidx{r}")
        nc.sync.dma_start(it_[:], inp[f"eidx_{r}"])
        eidx_t[r] = it_
        dt_ = wp.tile([P, NTILES], bf, tag=f"dl{r}", name=f"dl{r}")
        nc.sync.dma_start(dt_[:], inp[f"dl_{r}"])
        dl_t[r] = dt_
        qt_dram[r] = dp.tile([NSHP, C], bf, tag=f"qtd{r}", name=f"qtd{r}")
        kv_dram[r] = dp.tile([UCAP, 2 * C], bf, tag=f"kvd{r}", name=f"kvd{r}")

    # ---- stage 0: fuse relation transforms into projection weights ----
    Wv, Wq = {}, {}
    for r, T, S in RELS:
        for nm, Wd, WT, rel_w in (("v", Wv, vWT[S], mrel[r]),
                                  ("q", Wq, qWT[T], arelT[r])):
            Wt = wp.tile([P, 4, C], bf, tag=f"W{nm}{r}", name=f"W{nm}{r}")
            for cc in range(4):
                ps = pp.tile([P, C], f32, tag="ps", name="ps")
                for h in range(H):
                    nc.tensor.matmul(
                        ps[:, h * D:(h + 1) * D],
                        WT[:, h, cc * P:(cc + 1) * P],
                        rel_w[:, h * D:(h + 1) * D],
                        start=True, stop=True)
                nc.scalar.copy(Wt[:, cc, :], ps[:])
            Wd[r] = Wt

    qbr, vbr = {}, {}
    for r, T, S in RELS:
        for dd, src_b, rel_w in ((vbr, vb.get(S), mrel[r]),
                                 (qbr, qb.get(T), arelT[r])):
            if src_b is None:
                continue
            ps = pp.tile([P, C], f32, tag="ps", name="ps")
            for h in range(H):
                nc.tensor.matmul(ps[:1, h * D:(h + 1) * D],
                                 src_b[:, h:h + 1],
                                 rel_w[:, h * D:(h + 1) * D],
                                 start=True, stop=True)
            sb_ = wp.tile([1, C], f32, tag=f"bs{r}{len(dd)}", name=f"bs{r}{len(dd)}")
            nc.vector.tensor_copy(sb_[:], ps[:1, :])
            rep = wp.tile([P, C], f32, tag=f"br{r}{len(dd)}", name=f"br{r}{len(dd)}")
            nc.gpsimd.partition_broadcast(rep[:], sb_[:])
            dd[r] = rep

    # ---- stage 1: q~ tables (own dst shard; reads xsh only) ----
    for r, T, S in RELS:
        tb = 0 if T == "A" else NSHP
        nt_list = [4] * (NSHP // 512) + ([1] if (NSHP % 512) else [])
        i0 = 0
        for ntile in nt_list:
            w = ntile * P
            xs = sp.tile([P, 4, 512], bf, tag="xq", name="xq")
            for cc in range(4):
                nc.sync.dma_start(
                    xs[:, cc, :w],
                    inp["xsh"][tb + i0: tb + i0 + w, cc * P:(cc + 1) * P],
                    transpose=True)
            qs = sp.tile([P, 4, C], bf, tag="qs", name="qs")
            for t in range(ntile):
                ps = pp.tile([P, C], f32, tag="ps", name="ps")
                for cc in range(4):
                    nc.tensor.matmul(ps[:], xs[:, cc, t * P:(t + 1) * P],
                                     Wq[r][:, cc, :], start=(cc == 0), stop=(cc == 3))
                if r in qbr:
                    nc.vector.tensor_tensor(qs[:, t, :], ps[:], qbr[r][:], OP.add)
                else:
                    nc.scalar.copy(qs[:, t, :], ps[:])
            nc.sync.dma_start(
                qt_dram[r][:][i0:i0 + w, :].rearrange("(g p) c -> p g c", p=P),
                qs[:, :ntile, :])
            i0 += w

    # ---- stage 2: kv tables (compact unique sources from xfull) ----
    for r, T, S in RELS:
        soff = 0 if S == "A" else NSHP
        for o in range(NCORES):
            obase = o * XROWS + soff
            for ck in range(UCHUNK):
                row0 = (o * UCHUNK + ck) * 512
                xs = sp.tile([P, 4, 512], bf, tag="xs2", name="xs2")
                nc.gpsimd.dma_gather(
                    xs[:], xfull[:][obase:obase + NSHP, :],
                    uidx_t[r][:, row0 // 16:(row0 + 512) // 16],
                    512, 512, 512, transpose=True)
                kvs = sp.tile([P, 4, 2 * C], bf, tag="kvs", name="kvs")
                for t in range(4):
                    psk = pp.tile([P, C], f32, tag="ps", name="ps")
                    for cc in range(4):
                        nc.tensor.matmul(psk[:], xs[:, cc, t * P:(t + 1) * P],
                                         kW[S][:, cc, :], start=(cc == 0), stop=(cc == 3))
                    if S in kb:
                        nc.vector.tensor_tensor(kvs[:, t, 0:C], psk[:], kb[S][:], OP.add)
                    else:
                        nc.scalar.copy(kvs[:, t, 0:C], psk[:])
                    psv = pp.tile([P, C], f32, tag="ps", name="ps")
                    for cc in range(4):
                        nc.tensor.matmul(psv[:], xs[:, cc, t * P:(t + 1) * P],
                                         Wv[r][:, cc, :], start=(cc == 0), stop=(cc == 3))
                    if r in vbr:
                        nc.vector.tensor_tensor(kvs[:, t, C:], psv[:], vbr[r][:], OP.add)
                    else:
                        nc.scalar.copy(kvs[:, t, C:], psv[:])
                nc.sync.dma_start(
                    kv_dram[r][:][row0:row0 + 512, :].rearrange("(g p) c -> p g c", p=P),
                    kvs[:])

    # ---- stage 3: edge phase + output, per dst block ----
    for T, rels in TYPE_RELS:
        toff = 0 if T == "A" else NSHP
        for blk in range(NBLK):
            xo = ep.tile([P, C], bf, tag="xo", name="xo")
            nc.sync.dma_start(xo[:], inp["xsh"][toff + blk * P: toff + (blk + 1) * P, :])
            norms = []
            for r in rels:
                kv = gp.tile([P, TB, 2 * C], bf, tag="kv", name="kv")
                nc.gpsimd.dma_gather(
                    kv[:], kv_dram[r][:],
                    eidx_t[r][:, blk * TB * 8:(blk + 1) * TB * 8],
                    TB * P, TB * P, 2 * C)
                qt = ep.tile([P, C], bf, tag="qt", name="qt")
                nc.sync.dma_start(qt[:], qt_dram[r][:][blk * P:(blk + 1) * P, :])
                agg = agp.tile([P, C], f32, tag="agg", name="agg")
                den = dnp.tile([P, H], f32, tag="den", name="den")
                for t in range(TB):
                    mtT = ep.tile([P, P], bf, tag="mtT", name="mtT")
                    nc.vector.tensor_tensor(
                        mtT[:], iota[:],
                        dl_t[r][:, blk * TB + t:blk * TB + t + 1].to_broadcast([P, P]),
                        OP.is_equal)
                    mps = pp.tile([P, P], bf, tag="ps", name="ps")
                    nc.tensor.transpose(mps[:], mtT[:], ident[:])
                    mt = ep.tile([P, P], bf, tag="mt", name="mt")
                    nc.scalar.copy(mt[:], mps[:])
                    qe = pp.tile([P, C], f32, tag="ps", name="ps")
                    nc.tensor.matmul(qe[:], mt[:], qt[:], start=True, stop=True)
                    qeb = ep.tile([P, C], bf, tag="qeb", name="qeb")
                    nc.scalar.copy(qeb[:], qe[:])
                    prod = ep.tile([P, C], bf, tag="prod", name="prod")
                    nc.vector.tensor_tensor(prod[:], kv[:, t, 0:C], qeb[:], OP.mult)
                    L = ep.tile([P, H], f32, tag="L", name="L")
                    nc.vector.tensor_reduce(
                        L[:], prod[:].rearrange("p (h c) -> p h c", h=H),
                        axis=mybir.AxisListType.X, op=OP.add)
                    aT = ep.tile([P, H], bf, tag="aT", name="aT")
                    nc.scalar.activation(aT[:], L[:], AF.Exp)
                    va = ep.tile([P, C], bf, tag="va", name="va")
                    nc.vector.tensor_tensor(
                        va[:].rearrange("p (h c) -> p h c", h=H),
                        kv[:, t, C:].rearrange("p (h c) -> p h c", h=H),
                        aT[:].rearrange("p (h o) -> p h o", o=1).to_broadcast([P, H, D]),
                        OP.mult)
                    nc.tensor.matmul(agg[:], mtT[:], va[:],
                                     start=(t == 0), stop=(t == TB - 1))
                    nc.tensor.matmul(den[:], mtT[:], aT[:],
                                     start=(t == 0), stop=(t == TB - 1))
                dn = ep.tile([P, H], f32, tag="dn", name="dn")
                nc.vector.tensor_scalar_add(dn[:], den[:], 1e-16)
                rec = ep.tile([P, H], f32, tag="rec", name="rec")
                nc.vector.reciprocal(rec[:], dn[:])
                nrm = ep.tile([P, C], f32 if len(rels) > 1 else bf,
                              tag=f"nrm{len(norms)}", name=f"nrm{len(norms)}")
                nc.vector.tensor_tensor(
                    nrm[:].rearrange("p (h c) -> p h c", h=H),
                    agg[:].rearrange("p (h c) -> p h c", h=H),
                    rec[:].rearrange("p (h o) -> p h o", o=1).to_broadcast([P, H, D]),
                    OP.mult)
                norms.append(nrm)
            if len(norms) > 1:
                gsum = ep.tile([P, C], bf, tag="gsum", name="gsum")
                nc.vector.tensor_tensor(gsum[:], norms[0][:], norms[1][:], OP.add)
            else:
                gsum = norms[0]
            gel = ep.tile([P, C], bf, tag="gel", name="gel")
            nc.scalar.activation(gel[:], gsum[:], AF.Gelu)
            gT = ep.tile([P, 4, P], bf, tag="gT", name="gT")
            for cc in range(4):
                tp = pp.tile([P, P], bf, tag="ps", name="ps")
                nc.tensor.transpose(tp[:], gel[:, cc * P:(cc + 1) * P], ident[:])
                nc.scalar.copy(gT[:, cc, :], tp[:])
            o_ps = pp.tile([P, C], f32, tag="ps", name="ps")
            for cc in range(4):
                nc.tensor.matmul(o_ps[:], gT[:, cc, :], oW[T][:, cc, :],
                                 start=(cc == 0), stop=(cc == 3))
            if T in ob:
                nc.vector.tensor_tensor(o_ps[:], o_ps[:], ob[T][:], OP.add)
            xg = ep.tile([P, C], f32, tag="xg", name="xg")
            nc.scalar.activation(xg[:], xo[:], AF.Copy, scale=gate1m[T][:])
            hb = ep.tile([P, C], bf, tag="hb", name="hb")
            nc.vector.scalar_tensor_tensor(hb[:], o_ps[:], gate[T][:], xg[:],
                                           OP.mult, OP.add)
            hT = ep.tile([P, 4, P], bf, tag="hT", name="hT")
            for cc in range(4):
                tp = pp.tile([P, P], bf, tag="ps", name="ps")
                nc.tensor.transpose(tp[:], hb[:, cc * P:(cc + 1) * P], ident[:])
                nc.scalar.copy(hT[:, cc, :], tp[:])
            fin = pp.tile([P, 128], f32, tag="ps", name="ps")
            for cc in range(4):
                nc.tensor.matmul(fin[:], hT[:, cc, :], linW[:, cc, :],
                                 start=(cc == 0), stop=(cc == 3))
            fo = ep.tile([P, 128], bf, tag="fo", name="fo")
            if linb is not None:
                nc.vector.tensor_tensor(fo[:], fin[:], linb[:], OP.add)
            else:
                nc.scalar.copy(fo[:], fin[:])
            nc.sync.dma_start(out[toff + blk * P: toff + (blk + 1) * P, :], fo[:])


# ---------------------------------------------------------------------------
# Entry point
# ---------------------------------------------------------------------------

_CACHE = {}


def kernel(**inputs):
    inp = {k: np.asarray(v) for k, v in inputs.items()}
    shared = _prep_shared(inp)
    bz = {k: not np.any(np.asarray(inp[k])) for k in
          ("kb_A", "kb_B", "ob_A", "ob_B", "linb",
           "qb_A", "qb_B", "vb_A", "vb_B")}
    for k, z in bz.items():
        if z:
            shared.pop(k, None)
    key = tuple(sorted(bz.items()))
    if key not in _CACHE:
        _CACHE[key] = _build(bz)
    nc = _CACHE[key]

    in_maps = []
    for core in range(NCORES):
        m = dict(shared)
        m.update(_prep_core(core, inp))
        in_maps.append(m)

    import time as _time
    _t0 = _time.time()
    res = run_bass_kernel_spmd(nc, in_maps, core_ids=list(range(NCORES)))
    kernel.last_run_s = _time.time() - _t0
    kernel.last_results = res

    full = np.zeros((2 * N, 128), np.float32)
    for core in range(NCORES):
        o = res.results[core]["out"].astype(np.float32)
        full[core * NSH:(core + 1) * NSH] = o[:NSH]
        full[N + core * NSH:N + (core + 1) * NSH] = o[NSHP:NSHP + NSH]
    return full


# revision 4
# speedup vs baseline: 4.6449x; 1.0478x over previous
"""Distributed HGT message-passing kernel for 8 Trainium2 NeuronCores.

Wire-optimized dst-sharded design (host->device transfer dominates
end-to-end time, so inputs are kept minimal and everything derivable is
built on device):
  - Each core ships three packed blobs: a bf16 blob (its own dst shard
    of x + dst-local ids + iota const), an int16 blob (unreplicated
    gather indices, replicated across partitions on device), and a 1/8
    shard of the replicated weights (bf16).
  - On-device AllGathers assemble the full node-feature table and the
    full weight set; each core then gathers the unique boundary source
    features its edges reference (transposed dma_gather, per-owner
    bucket bases so indices fit int16) and builds compact
    kv_r = [k_raw || v @ mrel_r] tables.
  - One-hot dst masks for the edge phase are built on device from the
    dst-local ids (iota compare + PE transpose), not shipped.
  - The relation key-transform is folded into the query side
    (q~ = q @ arel^T * prel/sqrt(D)); softmax skips max-subtraction
    (logits are O(1)); exp runs in fp32.  Skip path and final output
    are bf16 (host upcasts).
"""

import math
import sys
from contextlib import ExitStack

import numpy as np
import ml_dtypes

sys.path.insert(0, "/opt/trn_rl_repo")

import concourse.bass as bass  # noqa: E402,F401
import concourse.mybir as mybir  # noqa: E402
import concourse.tile as tile  # noqa: E402
from concourse import bacc  # noqa: E402
from concourse.bass_utils import run_bass_kernel_spmd  # noqa: E402
from concourse.masks import make_identity  # noqa: E402

BF16 = ml_dtypes.bfloat16
N, E, C, H, D = 50000, 200000, 512, 8, 64
NCORES = 8
NSH = N // NCORES          # 6250 dst nodes per type per core
P = 128
NBLK = (NSH + P - 1) // P  # 49 dst blocks
NSHP = NBLK * P            # 6272 padded
XROWS = 2 * NSHP           # 12544 rows per core in the x shard (A then B)
XFULL = NCORES * XROWS     # 100352 rows after AllGather
TB = 5                     # edge tiles per dst block (640 edge slots)
NTILES = NBLK * TB         # 245 tiles per relation per core
UCB = 3072                 # compact-table rows per owner bucket (6 x 512)
UCHUNK = UCB // 512        # gather chunks per bucket
UCAP = NCORES * UCB        # 24576 compact rows per relation (< int16 max)
RELS = [("r1", "B", "A"), ("r2", "A", "B"), ("r3", "A", "A")]
TYPE_RELS = [("B", ["r1"]), ("A", ["r2", "r3"])]

f32 = mybir.dt.float32
bf = mybir.dt.bfloat16
i16 = mybir.dt.int16
AF = mybir.ActivationFunctionType
OP = mybir.AluOpType

# ---- packed-blob layouts (element offsets, all static) ----
W_MANIFEST = [
    ("kW_A", (4, P, C)), ("kW_B", (4, P, C)),
    ("qWT_A", (8, D, C)), ("qWT_B", (8, D, C)),
    ("vWT_A", (8, D, C)), ("vWT_B", (8, D, C)),
    ("oW_A", (4, P, C)), ("oW_B", (4, P, C)),
    ("linW", (4, P, 128)),
    ("mrel_r1", (D, C)), ("arelT_r1", (D, C)),
    ("mrel_r2", (D, C)), ("arelT_r2", (D, C)),
    ("mrel_r3", (D, C)), ("arelT_r3", (D, C)),
]
W_OFF = {}
_o = 0
for _nm, _sh in W_MANIFEST:
    W_OFF[_nm] = _o
    _o += int(np.prod(_sh))
W_TOTAL = _o
WS = -(-W_TOTAL // NCORES)          # per-core weight-shard elements
W_PAD = WS * NCORES

X_XSH = 0
X_DL = {r: XROWS * C + i * P * NTILES for i, (r, _, _) in enumerate(RELS)}
X_IOTA = XROWS * C + 3 * P * NTILES
XB = X_IOTA + P * P

I_UID = {r: i * (UCAP // 16) * 16 for i, (r, _, _) in enumerate(RELS)}
_ib = 3 * UCAP
I_EID = {r: _ib + i * NTILES * 8 * 16 for i, (r, _, _) in enumerate(RELS)}
IB = _ib + 3 * NTILES * P


# ---------------------------------------------------------------------------
# Host-side preprocessing (index routing + layout staging only)
# ---------------------------------------------------------------------------

def _wrap16(flat):
    """[n] -> [16, n//16] gather-index layout (16-partition wrap)."""
    return np.ascontiguousarray(flat.reshape(-1, 16).T)


def _prep_core(core, inp):
    lo = core * NSH
    xb = np.zeros(XB, BF16)
    xsh = xb[X_XSH:X_XSH + XROWS * C].reshape(XROWS, C)
    xsh[:NSH] = inp["x_A"][lo:lo + NSH].astype(BF16)
    xsh[NSHP:NSHP + NSH] = inp["x_B"][lo:lo + NSH].astype(BF16)
    xb[X_IOTA:X_IOTA + P * P] = np.tile(
        np.arange(P, dtype=np.float32).astype(BF16), P)

    ib = np.zeros(IB, np.int16)
    for r, T, S in RELS:
        ei = inp[f"ei_{r}"]
        src, dst = ei[0], ei[1]
        sel = (dst >= lo) & (dst < lo + NSH)
        src, dst = src[sel], dst[sel] - lo

        usrc, pos = np.unique(src, return_inverse=True)
        owner = usrc // NSH
        bcnt = np.bincount(owner, minlength=NCORES)
        assert bcnt.max() <= UCB, bcnt.max()
        boff = np.zeros(NCORES + 1, np.int64)
        boff[1:] = np.cumsum(bcnt)
        crow = owner * UCB + (np.arange(len(usrc)) - boff[owner])
        ulocal = np.zeros(UCAP, np.int64)
        ulocal[crow] = usrc % NSH
        ib[I_UID[r]:I_UID[r] + UCAP] = _wrap16(ulocal.astype(np.int16)).ravel()

        blk = dst // P
        cnt = np.bincount(blk, minlength=NBLK)
        assert cnt.max() <= TB * P, cnt.max()
        order = np.argsort(blk, kind="stable")
        epos = crow[pos][order]
        dloc = (dst[order] % P)

        eidx_flat = np.zeros(NTILES * P, np.int16)
        dl_flat = np.full(NTILES * P, 200.0, np.float64)  # 200 => empty slot
        off = 0
        for b in range(NBLK):
            nb_e = cnt[b]
            base = b * TB * P
            eidx_flat[base:base + nb_e] = epos[off:off + nb_e]
            dl_flat[base:base + nb_e] = dloc[off:off + nb_e]
            off += nb_e
        ib[I_EID[r]:I_EID[r] + NTILES * P] = _wrap16(eidx_flat).ravel()
        xb[X_DL[r]:X_DL[r] + P * NTILES] = np.ascontiguousarray(
            dl_flat.reshape(NTILES, P).T).astype(BF16).ravel()
    return {"xblob": xb, "iblob": ib}


def _prep_shared(inp):
    m = {}
    sD = 1.0 / math.sqrt(D)
    w = {}
    for t in ("A", "B"):
        w[f"kW_{t}"] = inp[f"kW_{t}"].reshape(4, P, C).astype(BF16)
        w[f"vWT_{t}"] = np.ascontiguousarray(inp[f"vW_{t}"].T).reshape(8, D, C).astype(BF16)
        w[f"qWT_{t}"] = np.ascontiguousarray(inp[f"qW_{t}"].T).reshape(8, D, C).astype(BF16)
        w[f"oW_{t}"] = inp[f"oW_{t}"].reshape(4, P, C).astype(BF16)
    w["linW"] = inp["linW"].reshape(4, P, 128).astype(BF16)
    for r, _, _ in RELS:
        w[f"mrel_{r}"] = np.ascontiguousarray(
            inp[f"mrel_{r}"].transpose(1, 0, 2)).reshape(D, C).astype(BF16)
        at = inp[f"arel_{r}"] * (inp[f"prel_{r}"] * sD)[:, None, None]
        w[f"arelT_{r}"] = np.ascontiguousarray(
            at.transpose(2, 0, 1)).reshape(D, C).astype(BF16)
    wflat = np.zeros(W_PAD, BF16)
    for nm, sh in W_MANIFEST:
        o = W_OFF[nm]
        wflat[o:o + int(np.prod(sh))] = w[nm].ravel()
    m["_wflat"] = wflat
    m["skp"] = np.array([[float(inp["skip_A"])], [float(inp["skip_B"])]], np.float32)
    for nm in ("kb_A", "kb_B", "ob_A", "ob_B"):
        m[nm] = np.asarray(inp[nm], np.float32).reshape(1, C)
    m["linb"] = np.asarray(inp["linb"], np.float32).reshape(1, 128)
    for t in ("A", "B"):
        for pfx in ("q", "v"):
            m[f"{pfx}b_{t}"] = np.ascontiguousarray(
                np.asarray(inp[f"{pfx}b_{t}"], np.float32).reshape(8, D).T)
    return m


# ---------------------------------------------------------------------------
# Device program
# ---------------------------------------------------------------------------

def _build(bz):
    nc = bacc.Bacc("TRN2", target_bir_lowering=False, debug=False,
                   enable_asserts=False, num_devices=NCORES)
    inp = {}

    def di(name, shape, dt):
        inp[name] = nc.dram_tensor(name, shape, dt, kind="ExternalInput").ap()

    di("xblob", [XB], bf)
    di("iblob", [IB], i16)
    di("wshard", [WS], bf)
    di("skp", [2, 1], f32)
    for t in ("A", "B"):
        if not bz[f"kb_{t}"]:
            di(f"kb_{t}", [1, C], f32)
        if not bz[f"ob_{t}"]:
            di(f"ob_{t}", [1, C], f32)
        if not bz[f"qb_{t}"]:
            di(f"qb_{t}", [D, 8], f32)
        if not bz[f"vb_{t}"]:
            di(f"vb_{t}", [D, 8], f32)
    if not bz["linb"]:
        di("linb", [1, 128], f32)
    out = nc.dram_tensor("out", [XROWS, 128], bf, kind="ExternalOutput").ap()

    with tile.TileContext(nc) as tc:
        with ExitStack() as es:
            _program(es, tc, inp, out, bz)
    nc.compile()
    return nc


def _program(es, tc, inp, out, bz):
    nc = tc.nc
    wp = es.enter_context(tc.tile_pool(name="w", bufs=1))
    dp = es.enter_context(tc.tile_pool(name="d", bufs=1, space="DRAM"))
    sp = es.enter_context(tc.tile_pool(name="s", bufs=2))
    ep = es.enter_context(tc.tile_pool(name="e", bufs=2))
    gp = es.enter_context(tc.tile_pool(name="g", bufs=2))
    pp = es.enter_context(tc.tile_pool(name="p", bufs=3, space="PSUM"))
    agp = es.enter_context(tc.tile_pool(name="a", bufs=2, space="PSUM"))
    dnp = es.enter_context(tc.tile_pool(name="n", bufs=2, space="PSUM"))

    xsh = inp["xblob"][X_XSH:X_XSH + XROWS * C].rearrange("(r c) -> r c", c=C)

    ident = wp.tile([P, P], bf, tag="ident", name="ident")
    make_identity(nc, ident[:])
    iota = wp.tile([P, P], bf, tag="iota", name="iota")
    nc.sync.dma_start(
        iota[:], inp["xblob"][X_IOTA:X_IOTA + P * P].rearrange("(p q) -> p q", q=P))

    # ---- all-gather x shards and weight shards ----
    xint = dp.tile([XROWS, C], bf, tag="xint", name="xint")
    xfull = dp.tile([XFULL, C], bf, tag="xfull", name="xfull",
                    addr_space="Shared")
    nc.gpsimd.dma_start(xint[:], xsh)
    nc.gpsimd.collective_compute(
        "AllGather", OP.bypass,
        replica_groups=[list(range(NCORES))],
        ins=[xint.opt()],
        outs=[xfull.opt()],
    )
    wint = dp.tile([1, WS], bf, tag="wint", name="wint")
    wfull = dp.tile([NCORES, WS], bf, tag="wfull", name="wfull",
                    addr_space="Shared")
    nc.gpsimd.dma_start(wint[:], inp["wshard"].rearrange("(o s) -> o s", o=1))
    nc.gpsimd.collective_compute(
        "AllGather", OP.bypass,
        replica_groups=[list(range(NCORES))],
        ins=[wint.opt()],
        outs=[wfull.opt()],
    )
    wflat = wfull[:].rearrange("o s -> (o s)")

    def load_w(name, shape, pat, **axes):
        t = wp.tile(list(shape), bf, tag=name, name=name)
        o = W_OFF[name]
        nc.sync.dma_start(
            t[:], wflat[o:o + int(np.prod(shape))].rearrange(pat, **axes))
        return t

    kW = {t: load_w(f"kW_{t}", (P, 4, C), "(c p o) -> p c o", c=4, p=P)
          for t in ("A", "B")}
    qWT = {t: load_w(f"qWT_{t}", (D, 8, C), "(h p o) -> p h o", h=8, p=D)
           for t in ("A", "B")}
    vWT = {t: load_w(f"vWT_{t}", (D, 8, C), "(h p o) -> p h o", h=8, p=D)
           for t in ("A", "B")}
    oW = {t: load_w(f"oW_{t}", (P, 4, C), "(c p o) -> p c o", c=4, p=P)
          for t in ("A", "B")}
    linW = load_w("linW", (P, 4, 128), "(c p o) -> p c o", c=4, p=P)
    mrel, arelT = {}, {}
    for r, _, _ in RELS:
        mrel[r] = load_w(f"mrel_{r}", (D, C), "(d c) -> d c", d=D)
        arelT[r] = load_w(f"arelT_{r}", (D, C), "(d c) -> d c", d=D)

    kb, ob, qb, vb, gate, gate1m = {}, {}, {}, {}, {}, {}
    for ti, t in enumerate(("A", "B")):
        for pfx, dd in (("kb", kb), ("ob", ob)):
            if not bz[f"{pfx}_{t}"]:
                row = wp.tile([1, C], f32, tag=f"{pfx}{t}r", name=f"{pfx}{t}r")
                nc.sync.dma_start(row[:], inp[f"{pfx}_{t}"])
                tt_ = wp.tile([P, C], f32, tag=f"{pfx}{t}", name=f"{pfx}{t}")
                nc.gpsimd.partition_broadcast(tt_[:], row[:])
                dd[t] = tt_
        for pfx, dd in (("qb", qb), ("vb", vb)):
            if not bz[f"{pfx}_{t}"]:
                tt_ = wp.tile([D, 8], f32, tag=f"{pfx}{t}", name=f"{pfx}{t}")
                nc.sync.dma_start(tt_[:], inp[f"{pfx}_{t}"])
                dd[t] = tt_
        sk = wp.tile([1, 1], f32, tag=f"sk{t}", name=f"sk{t}")
        nc.sync.dma_start(sk[:], inp["skp"][ti:ti + 1, :])
        g1_ = wp.tile([1, 1], f32, tag=f"g1r{t}", name=f"g1r{t}")
        nc.scalar.activation(g1_[:], sk[:], AF.Sigmoid)
        g_ = wp.tile([P, 1], f32, tag=f"g{t}", name=f"g{t}")
        nc.gpsimd.partition_broadcast(g_[:], g1_[:])
        gate[t] = g_
        g1 = wp.tile([P, 1], f32, tag=f"g1{t}", name=f"g1{t}")
        nc.vector.tensor_scalar(g1[:], g_[:], -1.0, 1.0, OP.mult, OP.add)
        gate1m[t] = g1
    linb = None
    if not bz["linb"]:
        lrow = wp.tile([1, 128], f32, tag="linbr", name="linbr")
        nc.sync.dma_start(lrow[:], inp["linb"])
        linb = wp.tile([P, 128], f32, tag="linb", name="linb")
        nc.gpsimd.partition_broadcast(linb[:], lrow[:])

    uidx_t, eidx_t, dl_t, qt_dram, kv_dram = {}, {}, {}, {}, {}
    for r, _, _ in RELS:
        ut_ = wp.tile([P, UCAP // 16], i16, tag=f"uidx{r}", name=f"uidx{r}")
        usrc_ap = inp["iblob"][I_UID[r]:I_UID[r] + UCAP].rearrange(
            "(p n) -> p n", p=16)
        for k in range(8):
            nc.sync.dma_start(ut_[16 * k:16 * (k + 1), :], usrc_ap)
        uidx_t[r] = ut_
        it_ = wp.tile([P, NTILES * 8], i16, tag=f"eidx{r}", name=f"eidx{r}")
        esrc_ap = inp["iblob"][I_EID[r]:I_EID[r] + NTILES * P].rearrange(
            "(p n) -> p n", p=16)
        for k in range(8):
            nc.sync.dma_start(it_[16 * k:16 * (k + 1), :], esrc_ap)
        eidx_t[r] = it_
        dt_ = wp.tile([P, NTILES], bf, tag=f"dl{r}", name=f"dl{r}")
        nc.sync.dma_start(
            dt_[:], inp["xblob"][X_DL[r]:X_DL[r] + P * NTILES].rearrange(
                "(p n) -> p n", n=NTILES))
        dl_t[r] = dt_
        qt_dram[r] = dp.tile([NSHP, C], bf, tag=f"qtd{r}", name=f"qtd{r}")
        kv_dram[r] = dp.tile([UCAP, 2 * C], bf, tag=f"kvd{r}", name=f"kvd{r}")

    # ---- stage 0: fuse relation transforms into projection weights ----
    Wv, Wq = {}, {}
    for r, T, S in RELS:
        for nm, Wd, WT, rel_w in (("v", Wv, vWT[S], mrel[r]),
                                  ("q", Wq, qWT[T], arelT[r])):
            Wt = wp.tile([P, 4, C], bf, tag=f"W{nm}{r}", name=f"W{nm}{r}")
            for cc in range(4):
                ps = pp.tile([P, C], f32, tag="ps", name="ps")
                for h in range(H):
                    nc.tensor.matmul(
                        ps[:, h * D:(h + 1) * D],
                        WT[:, h, cc * P:(cc + 1) * P],
                        rel_w[:, h * D:(h + 1) * D],
                        start=True, stop=True)
                nc.scalar.copy(Wt[:, cc, :], ps[:])
            Wd[r] = Wt

    qbr, vbr = {}, {}
    for r, T, S in RELS:
        for dd, src_b, rel_w in ((vbr, vb.get(S), mrel[r]),
                                 (qbr, qb.get(T), arelT[r])):
            if src_b is None:
                continue
            ps = pp.tile([P, C], f32, tag="ps", name="ps")
            for h in range(H):
                nc.tensor.matmul(ps[:1, h * D:(h + 1) * D],
                                 src_b[:, h:h + 1],
                                 rel_w[:, h * D:(h + 1) * D],
                                 start=True, stop=True)
            sb_ = wp.tile([1, C], f32, tag=f"bs{r}{len(dd)}", name=f"bs{r}{len(dd)}")
            nc.vector.tensor_copy(sb_[:], ps[:1, :])
            rep = wp.tile([P, C], f32, tag=f"br{r}{len(dd)}", name=f"br{r}{len(dd)}")
            nc.gpsimd.partition_broadcast(rep[:], sb_[:])
            dd[r] = rep

    # ---- stage 1: q~ tables (own dst shard; reads xsh only) ----
    for r, T, S in RELS:
        tb = 0 if T == "A" else NSHP
        nt_list = [4] * (NSHP // 512) + ([1] if (NSHP % 512) else [])
        i0 = 0
        for ntile in nt_list:
            w = ntile * P
            xs = sp.tile([P, 4, 512], bf, tag="xq", name="xq")
            for cc in range(4):
                nc.sync.dma_start(
                    xs[:, cc, :w],
                    xsh[tb + i0: tb + i0 + w, cc * P:(cc + 1) * P],
                    transpose=True)
            qs = sp.tile([P, 4, C], bf, tag="qs", name="qs")
            for t in range(ntile):
                ps = pp.tile([P, C], f32, tag="ps", name="ps")
                for cc in range(4):
                    nc.tensor.matmul(ps[:], xs[:, cc, t * P:(t + 1) * P],
                                     Wq[r][:, cc, :], start=(cc == 0), stop=(cc == 3))
                if r in qbr:
                    nc.vector.tensor_tensor(qs[:, t, :], ps[:], qbr[r][:], OP.add)
                else:
                    nc.scalar.copy(qs[:, t, :], ps[:])
            nc.sync.dma_start(
                qt_dram[r][:][i0:i0 + w, :].rearrange("(g p) c -> p g c", p=P),
                qs[:, :ntile, :])
            i0 += w

    # ---- stage 2: kv tables (compact unique sources from xfull) ----
    for r, T, S in RELS:
        soff = 0 if S == "A" else NSHP
        for o in range(NCORES):
            obase = o * XROWS + soff
            for ck in range(UCHUNK):
                row0 = (o * UCHUNK + ck) * 512
                xs = sp.tile([P, 4, 512], bf, tag="xs2", name="xs2")
                nc.gpsimd.dma_gather(
                    xs[:], xfull[:][obase:obase + NSHP, :],
                    uidx_t[r][:, row0 // 16:(row0 + 512) // 16],
                    512, 512, 512, transpose=True)
                kvs = sp.tile([P, 4, 2 * C], bf, tag="kvs", name="kvs")
                for t in range(4):
                    psk = pp.tile([P, C], f32, tag="ps", name="ps")
                    for cc in range(4):
                        nc.tensor.matmul(psk[:], xs[:, cc, t * P:(t + 1) * P],
                                         kW[S][:, cc, :], start=(cc == 0), stop=(cc == 3))
                    if S in kb:
                        nc.vector.tensor_tensor(kvs[:, t, 0:C], psk[:], kb[S][:], OP.add)
                    else:
                        nc.scalar.copy(kvs[:, t, 0:C], psk[:])
                    psv = pp.tile([P, C], f32, tag="ps", name="ps")
                    for cc in range(4):
                        nc.tensor.matmul(psv[:], xs[:, cc, t * P:(t + 1) * P],
                                         Wv[r][:, cc, :], start=(cc == 0), stop=(cc == 3))
                    if r in vbr:
                        nc.vector.tensor_tensor(kvs[:, t, C:], psv[:], vbr[r][:], OP.add)
                    else:
                        nc.scalar.copy(kvs[:, t, C:], psv[:])
                nc.sync.dma_start(
                    kv_dram[r][:][row0:row0 + 512, :].rearrange("(g p) c -> p g c", p=P),
                    kvs[:])

    # ---- stage 3: edge phase + output, per dst block ----
    for T, rels in TYPE_RELS:
        toff = 0 if T == "A" else NSHP
        for blk in range(NBLK):
            xo = ep.tile([P, C], bf, tag="xo", name="xo")
            nc.sync.dma_start(xo[:], xsh[toff + blk * P: toff + (blk + 1) * P, :])
            norms = []
            for r in rels:
                kv = gp.tile([P, TB, 2 * C], bf, tag="kv", name="kv")
                nc.gpsimd.dma_gather(
                    kv[:], kv_dram[r][:],
                    eidx_t[r][:, blk * TB * 8:(blk + 1) * TB * 8],
                    TB * P, TB * P, 2 * C)
                qt = ep.tile([P, C], bf, tag="qt", name="qt")
                nc.sync.dma_start(qt[:], qt_dram[r][:][blk * P:(blk + 1) * P, :])
                agg = agp.tile([P, C], f32, tag="agg", name="agg")
                den = dnp.tile([P, H], f32, tag="den", name="den")
                for t in range(TB):
                    mtT = ep.tile([P, P], bf, tag="mtT", name="mtT")
                    nc.vector.tensor_tensor(
                        mtT[:], iota[:],
                        dl_t[r][:, blk * TB + t:blk * TB + t + 1].to_broadcast([P, P]),
                        OP.is_equal)
                    mps = pp.tile([P, P], bf, tag="ps", name="ps")
                    nc.tensor.transpose(mps[:], mtT[:], ident[:])
                    mt = ep.tile([P, P], bf, tag="mt", name="mt")
                    nc.scalar.copy(mt[:], mps[:])
                    qe = pp.tile([P, C], f32, tag="ps", name="ps")
                    nc.tensor.matmul(qe[:], mt[:], qt[:], start=True, stop=True)
                    qeb = ep.tile([P, C], bf, tag="qeb", name="qeb")
                    nc.scalar.copy(qeb[:], qe[:])
                    prod = ep.tile([P, C], bf, tag="prod", name="prod")
                    nc.vector.tensor_tensor(prod[:], kv[:, t, 0:C], qeb[:], OP.mult)
                    L = ep.tile([P, H], f32, tag="L", name="L")
                    nc.vector.tensor_reduce(
                        L[:], prod[:].rearrange("p (h c) -> p h c", h=H),
                        axis=mybir.AxisListType.X, op=OP.add)
                    aT = ep.tile([P, H], bf, tag="aT", name="aT")
                    nc.scalar.activation(aT[:], L[:], AF.Exp)
                    va = ep.tile([P, C], bf, tag="va", name="va")
                    nc.vector.tensor_tensor(
                        va[:].rearrange("p (h c) -> p h c", h=H),
                        kv[:, t, C:].rearrange("p (h c) -> p h c", h=H),
                        aT[:].rearrange("p (h o) -> p h o", o=1).to_broadcast([P, H, D]),
                        OP.mult)
                    nc.tensor.matmul(agg[:], mtT[:], va[:],
                                     start=(t == 0), stop=(t == TB - 1))
                    nc.tensor.matmul(den[:], mtT[:], aT[:],
                                     start=(t == 0), stop=(t == TB - 1))
                dn = ep.tile([P, H], f32, tag="dn", name="dn")
                nc.vector.tensor_scalar_add(dn[:], den[:], 1e-16)
                rec = ep.tile([P, H], f32, tag="rec", name="rec")
                nc.vector.reciprocal(rec[:], dn[:])
                nrm = ep.tile([P, C], f32 if len(rels) > 1 else bf,
                              tag=f"nrm{len(norms)}", name=f"nrm{len(norms)}")
                nc.vector.tensor_tensor(
                    nrm[:].rearrange("p (h c) -> p h c", h=H),
                    agg[:].rearrange("p (h c) -> p h c", h=H),
                    rec[:].rearrange("p (h o) -> p h o", o=1).to_broadcast([P, H, D]),
                    OP.mult)
                norms.append(nrm)
            if len(norms) > 1:
                gsum = ep.tile([P, C], bf, tag="gsum", name="gsum")
                nc.vector.tensor_tensor(gsum[:], norms[0][:], norms[1][:], OP.add)
            else:
                gsum = norms[0]
            gel = ep.tile([P, C], bf, tag="gel", name="gel")
            nc.scalar.activation(gel[:], gsum[:], AF.Gelu)
            gT = ep.tile([P, 4, P], bf, tag="gT", name="gT")
            for cc in range(4):
                tp = pp.tile([P, P], bf, tag="ps", name="ps")
                nc.tensor.transpose(tp[:], gel[:, cc * P:(cc + 1) * P], ident[:])
                nc.scalar.copy(gT[:, cc, :], tp[:])
            o_ps = pp.tile([P, C], f32, tag="ps", name="ps")
            for cc in range(4):
                nc.tensor.matmul(o_ps[:], gT[:, cc, :], oW[T][:, cc, :],
                                 start=(cc == 0), stop=(cc == 3))
            if T in ob:
                nc.vector.tensor_tensor(o_ps[:], o_ps[:], ob[T][:], OP.add)
            xg = ep.tile([P, C], f32, tag="xg", name="xg")
            nc.scalar.activation(xg[:], xo[:], AF.Copy, scale=gate1m[T][:])
            hb = ep.tile([P, C], bf, tag="hb", name="hb")
            nc.vector.scalar_tensor_tensor(hb[:], o_ps[:], gate[T][:], xg[:],
                                           OP.mult, OP.add)
            hT = ep.tile([P, 4, P], bf, tag="hT", name="hT")
            for cc in range(4):
                tp = pp.tile([P, P], bf, tag="ps", name="ps")
                nc.tensor.transpose(tp[:], hb[:, cc * P:(cc + 1) * P], ident[:])
                nc.scalar.copy(hT[:, cc, :], tp[:])
            fin = pp.tile([P, 128], f32, tag="ps", name="ps")
            for cc in range(4):
                nc.tensor.matmul(fin[:], hT[:, cc, :], linW[:, cc, :],
                                 start=(cc == 0), stop=(cc == 3))
            fo = ep.tile([P, 128], bf, tag="fo", name="fo")
            if linb is not None:
                nc.vector.tensor_tensor(fo[:], fin[:], linb[:], OP.add)
            else:
                nc.scalar.copy(fo[:], fin[:])
            nc.sync.dma_start(out[toff + blk * P: toff + (blk + 1) * P, :], fo[:])


# ---------------------------------------------------------------------------
# Entry point
# ---------------------------------------------------------------------------

_CACHE = {}


def kernel(**inputs):
    inp = {k: np.asarray(v) for k, v in inputs.items()}
    shared = _prep_shared(inp)
    bz = {k: not np.any(np.asarray(inp[k])) for k in
          ("kb_A", "kb_B", "ob_A", "ob_B", "linb",
           "qb_A", "qb_B", "vb_A", "vb_B")}
    for k, z in bz.items():
        if z:
            shared.pop(k, None)
    wflat = shared.pop("_wflat")
    key = tuple(sorted(bz.items()))
    if key not in _CACHE:
        _CACHE[key] = _build(bz)
    nc = _CACHE[key]

    in_maps = []
    for core in range(NCORES):
        m = dict(shared)
        m["wshard"] = np.ascontiguousarray(wflat[core * WS:(core + 1) * WS])
        m.update(_prep_core(core, inp))
        in_maps.append(m)

    import time as _time
    _t0 = _time.time()
    res = run_bass_kernel_spmd(nc, in_maps, core_ids=list(range(NCORES)))
    kernel.last_run_s = _time.time() - _t0
    kernel.last_results = res

    full = np.zeros((2 * N, 128), np.float32)
    for core in range(NCORES):
        o = res.results[core]["out"].astype(np.float32)
        full[core * NSH:(core + 1) * NSH] = o[:NSH]
        full[N + core * NSH:N + (core + 1) * NSH] = o[NSHP:NSHP + NSH]
    return full


# revision 5
# speedup vs baseline: 8.1631x; 1.7574x over previous
"""Distributed HGT message-passing kernel for 8 Trainium2 NeuronCores.

Wire-optimized dst-sharded design (host->device transfer dominates
end-to-end time, so inputs are kept minimal and everything derivable is
built on device):
  - Each core ships three packed blobs: a bf16 blob (its own dst shard
    of x + dst-local ids + iota const), an int16 blob (unreplicated
    gather indices, replicated across partitions on device), and a 1/8
    shard of the replicated weights (bf16).
  - On-device AllGathers assemble the full node-feature table and the
    full weight set; each core then gathers the unique boundary source
    features its edges reference (transposed dma_gather, per-owner
    bucket bases so indices fit int16) and builds compact
    kv_r = [k_raw || v @ mrel_r] tables.
  - One-hot dst masks for the edge phase are built on device from the
    dst-local ids (iota compare + PE transpose), not shipped.
  - The relation key-transform is folded into the query side
    (q~ = q @ arel^T * prel/sqrt(D)); softmax skips max-subtraction
    (logits are O(1)); exp runs in fp32.  Skip path and final output
    are bf16 (host upcasts).
"""

import math
import sys
from contextlib import ExitStack

import numpy as np
import ml_dtypes

sys.path.insert(0, "/opt/trn_rl_repo")

# Persistent XLA compilation cache: run_bass_kernel_spmd re-jits a fresh
# closure every call, which otherwise re-runs the whole backend compile
# (incl. the BIR->NEFF hook) each time.  The disk cache makes repeat calls
# hit a deserialize instead.
import os as _os  # noqa: E402
import jax as _jax  # noqa: E402

_os.makedirs("/tmp/jax_bass_cache", exist_ok=True)
_jax.config.update("jax_compilation_cache_dir", "/tmp/jax_bass_cache")
_jax.config.update("jax_persistent_cache_min_compile_time_secs", 0)
_jax.config.update("jax_persistent_cache_min_entry_size_bytes", 0)

import concourse.bass as bass  # noqa: E402,F401
import concourse.mybir as mybir  # noqa: E402
import concourse.tile as tile  # noqa: E402
from concourse import bacc  # noqa: E402
from concourse.bass_utils import run_bass_kernel_spmd  # noqa: E402
from concourse.masks import make_identity  # noqa: E402

BF16 = ml_dtypes.bfloat16
N, E, C, H, D = 50000, 200000, 512, 8, 64
NCORES = 8
NSH = N // NCORES          # 6250 dst nodes per type per core
P = 128
NBLK = (NSH + P - 1) // P  # 49 dst blocks
NSHP = NBLK * P            # 6272 padded
XROWS = 2 * NSHP           # 12544 rows per core in the x shard (A then B)
XFULL = NCORES * XROWS     # 100352 rows after AllGather
TB = 5                     # edge tiles per dst block (640 edge slots)
NTILES = NBLK * TB         # 245 tiles per relation per core
UCB = 3072                 # compact-table rows per owner bucket (6 x 512)
UCHUNK = UCB // 512        # gather chunks per bucket
UCAP = NCORES * UCB        # 24576 compact rows per relation (< int16 max)
RELS = [("r1", "B", "A"), ("r2", "A", "B"), ("r3", "A", "A")]
TYPE_RELS = [("B", ["r1"]), ("A", ["r2", "r3"])]

f32 = mybir.dt.float32
bf = mybir.dt.bfloat16
i16 = mybir.dt.int16
AF = mybir.ActivationFunctionType
OP = mybir.AluOpType

# ---- packed-blob layouts (element offsets, all static) ----
W_MANIFEST = [
    ("kW_A", (4, P, C)), ("kW_B", (4, P, C)),
    ("qWT_A", (8, D, C)), ("qWT_B", (8, D, C)),
    ("vWT_A", (8, D, C)), ("vWT_B", (8, D, C)),
    ("oW_A", (4, P, C)), ("oW_B", (4, P, C)),
    ("linW", (4, P, 128)),
    ("mrel_r1", (D, C)), ("arelT_r1", (D, C)),
    ("mrel_r2", (D, C)), ("arelT_r2", (D, C)),
    ("mrel_r3", (D, C)), ("arelT_r3", (D, C)),
]
W_OFF = {}
_o = 0
for _nm, _sh in W_MANIFEST:
    W_OFF[_nm] = _o
    _o += int(np.prod(_sh))
W_TOTAL = _o
WS = -(-W_TOTAL // NCORES)          # per-core weight-shard elements
W_PAD = WS * NCORES

X_XSH = 0
X_DL = {r: XROWS * C + i * P * NTILES for i, (r, _, _) in enumerate(RELS)}
X_IOTA = XROWS * C + 3 * P * NTILES
XB = X_IOTA + P * P

I_UID = {r: i * (UCAP // 16) * 16 for i, (r, _, _) in enumerate(RELS)}
_ib = 3 * UCAP
I_EID = {r: _ib + i * NTILES * 8 * 16 for i, (r, _, _) in enumerate(RELS)}
IB = _ib + 3 * NTILES * P


# ---------------------------------------------------------------------------
# Host-side preprocessing (index routing + layout staging only)
# ---------------------------------------------------------------------------

def _wrap16(flat):
    """[n] -> [16, n//16] gather-index layout (16-partition wrap)."""
    return np.ascontiguousarray(flat.reshape(-1, 16).T)


def _prep_core(core, inp):
    lo = core * NSH
    xb = np.zeros(XB, BF16)
    xsh = xb[X_XSH:X_XSH + XROWS * C].reshape(XROWS, C)
    xsh[:NSH] = inp["x_A"][lo:lo + NSH].astype(BF16)
    xsh[NSHP:NSHP + NSH] = inp["x_B"][lo:lo + NSH].astype(BF16)
    xb[X_IOTA:X_IOTA + P * P] = np.tile(
        np.arange(P, dtype=np.float32).astype(BF16), P)

    ib = np.zeros(IB, np.int16)
    for r, T, S in RELS:
        ei = inp[f"ei_{r}"]
        src, dst = ei[0], ei[1]
        sel = (dst >= lo) & (dst < lo + NSH)
        src, dst = src[sel], dst[sel] - lo

        usrc, pos = np.unique(src, return_inverse=True)
        owner = usrc // NSH
        bcnt = np.bincount(owner, minlength=NCORES)
        assert bcnt.max() <= UCB, bcnt.max()
        boff = np.zeros(NCORES + 1, np.int64)
        boff[1:] = np.cumsum(bcnt)
        crow = owner * UCB + (np.arange(len(usrc)) - boff[owner])
        ulocal = np.zeros(UCAP, np.int64)
        ulocal[crow] = usrc % NSH
        ib[I_UID[r]:I_UID[r] + UCAP] = _wrap16(ulocal.astype(np.int16)).ravel()

        blk = dst // P
        cnt = np.bincount(blk, minlength=NBLK)
        assert cnt.max() <= TB * P, cnt.max()
        order = np.argsort(blk, kind="stable")
        epos = crow[pos][order]
        dloc = (dst[order] % P)

        eidx_flat = np.zeros(NTILES * P, np.int16)
        dl_flat = np.full(NTILES * P, 200.0, np.float64)  # 200 => empty slot
        off = 0
        for b in range(NBLK):
            nb_e = cnt[b]
            base = b * TB * P
            eidx_flat[base:base + nb_e] = epos[off:off + nb_e]
            dl_flat[base:base + nb_e] = dloc[off:off + nb_e]
            off += nb_e
        ib[I_EID[r]:I_EID[r] + NTILES * P] = _wrap16(eidx_flat).ravel()
        xb[X_DL[r]:X_DL[r] + P * NTILES] = np.ascontiguousarray(
            dl_flat.reshape(NTILES, P).T).astype(BF16).ravel()
    return {"xblob": xb, "iblob": ib}


def _prep_shared(inp):
    m = {}
    sD = 1.0 / math.sqrt(D)
    w = {}
    for t in ("A", "B"):
        w[f"kW_{t}"] = inp[f"kW_{t}"].reshape(4, P, C).astype(BF16)
        w[f"vWT_{t}"] = np.ascontiguousarray(inp[f"vW_{t}"].T).reshape(8, D, C).astype(BF16)
        w[f"qWT_{t}"] = np.ascontiguousarray(inp[f"qW_{t}"].T).reshape(8, D, C).astype(BF16)
        w[f"oW_{t}"] = inp[f"oW_{t}"].reshape(4, P, C).astype(BF16)
    w["linW"] = inp["linW"].reshape(4, P, 128).astype(BF16)
    for r, _, _ in RELS:
        w[f"mrel_{r}"] = np.ascontiguousarray(
            inp[f"mrel_{r}"].transpose(1, 0, 2)).reshape(D, C).astype(BF16)
        at = inp[f"arel_{r}"] * (inp[f"prel_{r}"] * sD)[:, None, None]
        w[f"arelT_{r}"] = np.ascontiguousarray(
            at.transpose(2, 0, 1)).reshape(D, C).astype(BF16)
    wflat = np.zeros(W_PAD, BF16)
    for nm, sh in W_MANIFEST:
        o = W_OFF[nm]
        wflat[o:o + int(np.prod(sh))] = w[nm].ravel()
    m["_wflat"] = wflat
    m["skp"] = np.array([[float(inp["skip_A"])], [float(inp["skip_B"])]], np.float32)
    for nm in ("kb_A", "kb_B", "ob_A", "ob_B"):
        m[nm] = np.asarray(inp[nm], np.float32).reshape(1, C)
    m["linb"] = np.asarray(inp["linb"], np.float32).reshape(1, 128)
    for t in ("A", "B"):
        for pfx in ("q", "v"):
            m[f"{pfx}b_{t}"] = np.ascontiguousarray(
                np.asarray(inp[f"{pfx}b_{t}"], np.float32).reshape(8, D).T)
    return m


# ---------------------------------------------------------------------------
# Device program
# ---------------------------------------------------------------------------

def _build(bz):
    nc = bacc.Bacc("TRN2", target_bir_lowering=False, debug=False,
                   enable_asserts=False, num_devices=NCORES)
    inp = {}

    def di(name, shape, dt):
        inp[name] = nc.dram_tensor(name, shape, dt, kind="ExternalInput").ap()

    di("xblob", [XB], bf)
    di("iblob", [IB], i16)
    di("wshard", [WS], bf)
    di("skp", [2, 1], f32)
    for t in ("A", "B"):
        if not bz[f"kb_{t}"]:
            di(f"kb_{t}", [1, C], f32)
        if not bz[f"ob_{t}"]:
            di(f"ob_{t}", [1, C], f32)
        if not bz[f"qb_{t}"]:
            di(f"qb_{t}", [D, 8], f32)
        if not bz[f"vb_{t}"]:
            di(f"vb_{t}", [D, 8], f32)
    if not bz["linb"]:
        di("linb", [1, 128], f32)
    out = nc.dram_tensor("out", [XROWS, 128], bf, kind="ExternalOutput").ap()

    with tile.TileContext(nc) as tc:
        with ExitStack() as es:
            _program(es, tc, inp, out, bz)
    nc.compile()
    return nc


def _program(es, tc, inp, out, bz):
    nc = tc.nc
    wp = es.enter_context(tc.tile_pool(name="w", bufs=1))
    dp = es.enter_context(tc.tile_pool(name="d", bufs=1, space="DRAM"))
    sp = es.enter_context(tc.tile_pool(name="s", bufs=2))
    ep = es.enter_context(tc.tile_pool(name="e", bufs=2))
    gp = es.enter_context(tc.tile_pool(name="g", bufs=2))
    pp = es.enter_context(tc.tile_pool(name="p", bufs=3, space="PSUM"))
    agp = es.enter_context(tc.tile_pool(name="a", bufs=2, space="PSUM"))
    dnp = es.enter_context(tc.tile_pool(name="n", bufs=2, space="PSUM"))

    xsh = inp["xblob"][X_XSH:X_XSH + XROWS * C].rearrange("(r c) -> r c", c=C)

    ident = wp.tile([P, P], bf, tag="ident", name="ident")
    make_identity(nc, ident[:])
    iota = wp.tile([P, P], bf, tag="iota", name="iota")
    nc.sync.dma_start(
        iota[:], inp["xblob"][X_IOTA:X_IOTA + P * P].rearrange("(p q) -> p q", q=P))

    # ---- all-gather x shards and weight shards ----
    xint = dp.tile([XROWS, C], bf, tag="xint", name="xint")
    xfull = dp.tile([XFULL, C], bf, tag="xfull", name="xfull",
                    addr_space="Shared")
    nc.gpsimd.dma_start(xint[:], xsh)
    nc.gpsimd.collective_compute(
        "AllGather", OP.bypass,
        replica_groups=[list(range(NCORES))],
        ins=[xint.opt()],
        outs=[xfull.opt()],
    )
    wint = dp.tile([1, WS], bf, tag="wint", name="wint")
    wfull = dp.tile([NCORES, WS], bf, tag="wfull", name="wfull",
                    addr_space="Shared")
    nc.gpsimd.dma_start(wint[:], inp["wshard"].rearrange("(o s) -> o s", o=1))
    nc.gpsimd.collective_compute(
        "AllGather", OP.bypass,
        replica_groups=[list(range(NCORES))],
        ins=[wint.opt()],
        outs=[wfull.opt()],
    )
    wflat = wfull[:].rearrange("o s -> (o s)")

    def load_w(name, shape, pat, **axes):
        t = wp.tile(list(shape), bf, tag=name, name=name)
        o = W_OFF[name]
        nc.sync.dma_start(
            t[:], wflat[o:o + int(np.prod(shape))].rearrange(pat, **axes))
        return t

    kW = {t: load_w(f"kW_{t}", (P, 4, C), "(c p o) -> p c o", c=4, p=P)
          for t in ("A", "B")}
    qWT = {t: load_w(f"qWT_{t}", (D, 8, C), "(h p o) -> p h o", h=8, p=D)
           for t in ("A", "B")}
    vWT = {t: load_w(f"vWT_{t}", (D, 8, C), "(h p o) -> p h o", h=8, p=D)
           for t in ("A", "B")}
    oW = {t: load_w(f"oW_{t}", (P, 4, C), "(c p o) -> p c o", c=4, p=P)
          for t in ("A", "B")}
    linW = load_w("linW", (P, 4, 128), "(c p o) -> p c o", c=4, p=P)
    mrel, arelT = {}, {}
    for r, _, _ in RELS:
        mrel[r] = load_w(f"mrel_{r}", (D, C), "(d c) -> d c", d=D)
        arelT[r] = load_w(f"arelT_{r}", (D, C), "(d c) -> d c", d=D)

    kb, ob, qb, vb, gate, gate1m = {}, {}, {}, {}, {}, {}
    for ti, t in enumerate(("A", "B")):
        for pfx, dd in (("kb", kb), ("ob", ob)):
            if not bz[f"{pfx}_{t}"]:
                row = wp.tile([1, C], f32, tag=f"{pfx}{t}r", name=f"{pfx}{t}r")
                nc.sync.dma_start(row[:], inp[f"{pfx}_{t}"])
                tt_ = wp.tile([P, C], f32, tag=f"{pfx}{t}", name=f"{pfx}{t}")
                nc.gpsimd.partition_broadcast(tt_[:], row[:])
                dd[t] = tt_
        for pfx, dd in (("qb", qb), ("vb", vb)):
            if not bz[f"{pfx}_{t}"]:
                tt_ = wp.tile([D, 8], f32, tag=f"{pfx}{t}", name=f"{pfx}{t}")
                nc.sync.dma_start(tt_[:], inp[f"{pfx}_{t}"])
                dd[t] = tt_
        sk = wp.tile([1, 1], f32, tag=f"sk{t}", name=f"sk{t}")
        nc.sync.dma_start(sk[:], inp["skp"][ti:ti + 1, :])
        g1_ = wp.tile([1, 1], f32, tag=f"g1r{t}", name=f"g1r{t}")
        nc.scalar.activation(g1_[:], sk[:], AF.Sigmoid)
        g_ = wp.tile([P, 1], f32, tag=f"g{t}", name=f"g{t}")
        nc.gpsimd.partition_broadcast(g_[:], g1_[:])
        gate[t] = g_
        g1 = wp.tile([P, 1], f32, tag=f"g1{t}", name=f"g1{t}")
        nc.vector.tensor_scalar(g1[:], g_[:], -1.0, 1.0, OP.mult, OP.add)
        gate1m[t] = g1
    linb = None
    if not bz["linb"]:
        lrow = wp.tile([1, 128], f32, tag="linbr", name="linbr")
        nc.sync.dma_start(lrow[:], inp["linb"])
        linb = wp.tile([P, 128], f32, tag="linb", name="linb")
        nc.gpsimd.partition_broadcast(linb[:], lrow[:])

    uidx_t, eidx_t, dl_t, qt_dram, kv_dram = {}, {}, {}, {}, {}
    for r, _, _ in RELS:
        ut_ = wp.tile([P, UCAP // 16], i16, tag=f"uidx{r}", name=f"uidx{r}")
        usrc_ap = inp["iblob"][I_UID[r]:I_UID[r] + UCAP].rearrange(
            "(p n) -> p n", p=16)
        for k in range(8):
            nc.sync.dma_start(ut_[16 * k:16 * (k + 1), :], usrc_ap)
        uidx_t[r] = ut_
        it_ = wp.tile([P, NTILES * 8], i16, tag=f"eidx{r}", name=f"eidx{r}")
        esrc_ap = inp["iblob"][I_EID[r]:I_EID[r] + NTILES * P].rearrange(
            "(p n) -> p n", p=16)
        for k in range(8):
            nc.sync.dma_start(it_[16 * k:16 * (k + 1), :], esrc_ap)
        eidx_t[r] = it_
        dt_ = wp.tile([P, NTILES], bf, tag=f"dl{r}", name=f"dl{r}")
        nc.sync.dma_start(
            dt_[:], inp["xblob"][X_DL[r]:X_DL[r] + P * NTILES].rearrange(
                "(p n) -> p n", n=NTILES))
        dl_t[r] = dt_
        qt_dram[r] = dp.tile([NSHP, C], bf, tag=f"qtd{r}", name=f"qtd{r}")
        kv_dram[r] = dp.tile([UCAP, 2 * C], bf, tag=f"kvd{r}", name=f"kvd{r}")

    # ---- stage 0: fuse relation transforms into projection weights ----
    Wv, Wq = {}, {}
    for r, T, S in RELS:
        for nm, Wd, WT, rel_w in (("v", Wv, vWT[S], mrel[r]),
                                  ("q", Wq, qWT[T], arelT[r])):
            Wt = wp.tile([P, 4, C], bf, tag=f"W{nm}{r}", name=f"W{nm}{r}")
            for cc in range(4):
                ps = pp.tile([P, C], f32, tag="ps", name="ps")
                for h in range(H):
                    nc.tensor.matmul(
                        ps[:, h * D:(h + 1) * D],
                        WT[:, h, cc * P:(cc + 1) * P],
                        rel_w[:, h * D:(h + 1) * D],
                        start=True, stop=True)
                nc.scalar.copy(Wt[:, cc, :], ps[:])
            Wd[r] = Wt

    qbr, vbr = {}, {}
    for r, T, S in RELS:
        for dd, src_b, rel_w in ((vbr, vb.get(S), mrel[r]),
                                 (qbr, qb.get(T), arelT[r])):
            if src_b is None:
                continue
            ps = pp.tile([P, C], f32, tag="ps", name="ps")
            for h in range(H):
                nc.tensor.matmul(ps[:1, h * D:(h + 1) * D],
                                 src_b[:, h:h + 1],
                                 rel_w[:, h * D:(h + 1) * D],
                                 start=True, stop=True)
            sb_ = wp.tile([1, C], f32, tag=f"bs{r}{len(dd)}", name=f"bs{r}{len(dd)}")
            nc.vector.tensor_copy(sb_[:], ps[:1, :])
            rep = wp.tile([P, C], f32, tag=f"br{r}{len(dd)}", name=f"br{r}{len(dd)}")
            nc.gpsimd.partition_broadcast(rep[:], sb_[:])
            dd[r] = rep

    # ---- stage 1: q~ tables (own dst shard; reads xsh only) ----
    for r, T, S in RELS:
        tb = 0 if T == "A" else NSHP
        nt_list = [4] * (NSHP // 512) + ([1] if (NSHP % 512) else [])
        i0 = 0
        for ntile in nt_list:
            w = ntile * P
            xs = sp.tile([P, 4, 512], bf, tag="xq", name="xq")
            for cc in range(4):
                nc.sync.dma_start(
                    xs[:, cc, :w],
                    xsh[tb + i0: tb + i0 + w, cc * P:(cc + 1) * P],
                    transpose=True)
            qs = sp.tile([P, 4, C], bf, tag="qs", name="qs")
            for t in range(ntile):
                ps = pp.tile([P, C], f32, tag="ps", name="ps")
                for cc in range(4):
                    nc.tensor.matmul(ps[:], xs[:, cc, t * P:(t + 1) * P],
                                     Wq[r][:, cc, :], start=(cc == 0), stop=(cc == 3))
                if r in qbr:
                    nc.vector.tensor_tensor(qs[:, t, :], ps[:], qbr[r][:], OP.add)
                else:
                    nc.scalar.copy(qs[:, t, :], ps[:])
            nc.sync.dma_start(
                qt_dram[r][:][i0:i0 + w, :].rearrange("(g p) c -> p g c", p=P),
                qs[:, :ntile, :])
            i0 += w

    # ---- stage 2: kv tables (compact unique sources from xfull) ----
    for r, T, S in RELS:
        soff = 0 if S == "A" else NSHP
        for o in range(NCORES):
            obase = o * XROWS + soff
            for ck in range(UCHUNK):
                row0 = (o * UCHUNK + ck) * 512
                xs = sp.tile([P, 4, 512], bf, tag="xs2", name="xs2")
                nc.gpsimd.dma_gather(
                    xs[:], xfull[:][obase:obase + NSHP, :],
                    uidx_t[r][:, row0 // 16:(row0 + 512) // 16],
                    512, 512, 512, transpose=True)
                kvs = sp.tile([P, 4, 2 * C], bf, tag="kvs", name="kvs")
                for t in range(4):
                    psk = pp.tile([P, C], f32, tag="ps", name="ps")
                    for cc in range(4):
                        nc.tensor.matmul(psk[:], xs[:, cc, t * P:(t + 1) * P],
                                         kW[S][:, cc, :], start=(cc == 0), stop=(cc == 3))
                    if S in kb:
                        nc.vector.tensor_tensor(kvs[:, t, 0:C], psk[:], kb[S][:], OP.add)
                    else:
                        nc.scalar.copy(kvs[:, t, 0:C], psk[:])
                    psv = pp.tile([P, C], f32, tag="ps", name="ps")
                    for cc in range(4):
                        nc.tensor.matmul(psv[:], xs[:, cc, t * P:(t + 1) * P],
                                         Wv[r][:, cc, :], start=(cc == 0), stop=(cc == 3))
                    if r in vbr:
                        nc.vector.tensor_tensor(kvs[:, t, C:], psv[:], vbr[r][:], OP.add)
                    else:
                        nc.scalar.copy(kvs[:, t, C:], psv[:])
                nc.sync.dma_start(
                    kv_dram[r][:][row0:row0 + 512, :].rearrange("(g p) c -> p g c", p=P),
                    kvs[:])

    # ---- stage 3: edge phase + output, per dst block ----
    for T, rels in TYPE_RELS:
        toff = 0 if T == "A" else NSHP
        for blk in range(NBLK):
            xo = ep.tile([P, C], bf, tag="xo", name="xo")
            nc.sync.dma_start(xo[:], xsh[toff + blk * P: toff + (blk + 1) * P, :])
            norms = []
            for r in rels:
                kv = gp.tile([P, TB, 2 * C], bf, tag="kv", name="kv")
                nc.gpsimd.dma_gather(
                    kv[:], kv_dram[r][:],
                    eidx_t[r][:, blk * TB * 8:(blk + 1) * TB * 8],
                    TB * P, TB * P, 2 * C)
                qt = ep.tile([P, C], bf, tag="qt", name="qt")
                nc.sync.dma_start(qt[:], qt_dram[r][:][blk * P:(blk + 1) * P, :])
                agg = agp.tile([P, C], f32, tag="agg", name="agg")
                den = dnp.tile([P, H], f32, tag="den", name="den")
                for t in range(TB):
                    mtT = ep.tile([P, P], bf, tag="mtT", name="mtT")
                    nc.vector.tensor_tensor(
                        mtT[:], iota[:],
                        dl_t[r][:, blk * TB + t:blk * TB + t + 1].to_broadcast([P, P]),
                        OP.is_equal)
                    mps = pp.tile([P, P], bf, tag="ps", name="ps")
                    nc.tensor.transpose(mps[:], mtT[:], ident[:])
                    mt = ep.tile([P, P], bf, tag="mt", name="mt")
                    nc.scalar.copy(mt[:], mps[:])
                    qe = pp.tile([P, C], f32, tag="ps", name="ps")
                    nc.tensor.matmul(qe[:], mt[:], qt[:], start=True, stop=True)
                    qeb = ep.tile([P, C], bf, tag="qeb", name="qeb")
                    nc.scalar.copy(qeb[:], qe[:])
                    prod = ep.tile([P, C], bf, tag="prod", name="prod")
                    nc.vector.tensor_tensor(prod[:], kv[:, t, 0:C], qeb[:], OP.mult)
                    L = ep.tile([P, H], f32, tag="L", name="L")
                    nc.vector.tensor_reduce(
                        L[:], prod[:].rearrange("p (h c) -> p h c", h=H),
                        axis=mybir.AxisListType.X, op=OP.add)
                    aT = ep.tile([P, H], bf, tag="aT", name="aT")
                    nc.scalar.activation(aT[:], L[:], AF.Exp)
                    va = ep.tile([P, C], bf, tag="va", name="va")
                    nc.vector.tensor_tensor(
                        va[:].rearrange("p (h c) -> p h c", h=H),
                        kv[:, t, C:].rearrange("p (h c) -> p h c", h=H),
                        aT[:].rearrange("p (h o) -> p h o", o=1).to_broadcast([P, H, D]),
                        OP.mult)
                    nc.tensor.matmul(agg[:], mtT[:], va[:],
                                     start=(t == 0), stop=(t == TB - 1))
                    nc.tensor.matmul(den[:], mtT[:], aT[:],
                                     start=(t == 0), stop=(t == TB - 1))
                dn = ep.tile([P, H], f32, tag="dn", name="dn")
                nc.vector.tensor_scalar_add(dn[:], den[:], 1e-16)
                rec = ep.tile([P, H], f32, tag="rec", name="rec")
                nc.vector.reciprocal(rec[:], dn[:])
                nrm = ep.tile([P, C], f32 if len(rels) > 1 else bf,
                              tag=f"nrm{len(norms)}", name=f"nrm{len(norms)}")
                nc.vector.tensor_tensor(
                    nrm[:].rearrange("p (h c) -> p h c", h=H),
                    agg[:].rearrange("p (h c) -> p h c", h=H),
                    rec[:].rearrange("p (h o) -> p h o", o=1).to_broadcast([P, H, D]),
                    OP.mult)
                norms.append(nrm)
            if len(norms) > 1:
                gsum = ep.tile([P, C], bf, tag="gsum", name="gsum")
                nc.vector.tensor_tensor(gsum[:], norms[0][:], norms[1][:], OP.add)
            else:
                gsum = norms[0]
            gel = ep.tile([P, C], bf, tag="gel", name="gel")
            nc.scalar.activation(gel[:], gsum[:], AF.Gelu)
            gT = ep.tile([P, 4, P], bf, tag="gT", name="gT")
            for cc in range(4):
                tp = pp.tile([P, P], bf, tag="ps", name="ps")
                nc.tensor.transpose(tp[:], gel[:, cc * P:(cc + 1) * P], ident[:])
                nc.scalar.copy(gT[:, cc, :], tp[:])
            o_ps = pp.tile([P, C], f32, tag="ps", name="ps")
            for cc in range(4):
                nc.tensor.matmul(o_ps[:], gT[:, cc, :], oW[T][:, cc, :],
                                 start=(cc == 0), stop=(cc == 3))
            if T in ob:
                nc.vector.tensor_tensor(o_ps[:], o_ps[:], ob[T][:], OP.add)
            xg = ep.tile([P, C], f32, tag="xg", name="xg")
            nc.scalar.activation(xg[:], xo[:], AF.Copy, scale=gate1m[T][:])
            hb = ep.tile([P, C], bf, tag="hb", name="hb")
            nc.vector.scalar_tensor_tensor(hb[:], o_ps[:], gate[T][:], xg[:],
                                           OP.mult, OP.add)
            hT = ep.tile([P, 4, P], bf, tag="hT", name="hT")
            for cc in range(4):
                tp = pp.tile([P, P], bf, tag="ps", name="ps")
                nc.tensor.transpose(tp[:], hb[:, cc * P:(cc + 1) * P], ident[:])
                nc.scalar.copy(hT[:, cc, :], tp[:])
            fin = pp.tile([P, 128], f32, tag="ps", name="ps")
            for cc in range(4):
                nc.tensor.matmul(fin[:], hT[:, cc, :], linW[:, cc, :],
                                 start=(cc == 0), stop=(cc == 3))
            fo = ep.tile([P, 128], bf, tag="fo", name="fo")
            if linb is not None:
                nc.vector.tensor_tensor(fo[:], fin[:], linb[:], OP.add)
            else:
                nc.scalar.copy(fo[:], fin[:])
            nc.sync.dma_start(out[toff + blk * P: toff + (blk + 1) * P, :], fo[:])


# ---------------------------------------------------------------------------
# Entry point
# ---------------------------------------------------------------------------

_CACHE = {}


def kernel(**inputs):
    inp = {k: np.asarray(v) for k, v in inputs.items()}
    shared = _prep_shared(inp)
    bz = {k: not np.any(np.asarray(inp[k])) for k in
          ("kb_A", "kb_B", "ob_A", "ob_B", "linb",
           "qb_A", "qb_B", "vb_A", "vb_B")}
    for k, z in bz.items():
        if z:
            shared.pop(k, None)
    wflat = shared.pop("_wflat")
    key = tuple(sorted(bz.items()))
    if key not in _CACHE:
        _CACHE[key] = _build(bz)
    nc = _CACHE[key]

    in_maps = []
    for core in range(NCORES):
        m = dict(shared)
        m["wshard"] = np.ascontiguousarray(wflat[core * WS:(core + 1) * WS])
        m.update(_prep_core(core, inp))
        in_maps.append(m)

    import time as _time
    _t0 = _time.time()
    res = run_bass_kernel_spmd(nc, in_maps, core_ids=list(range(NCORES)))
    kernel.last_run_s = _time.time() - _t0
    kernel.last_results = res

    full = np.zeros((2 * N, 128), np.float32)
    for core in range(NCORES):
        o = res.results[core]["out"].astype(np.float32)
        full[core * NSH:(core + 1) * NSH] = o[:NSH]
        full[N + core * NSH:N + (core + 1) * NSH] = o[NSHP:NSHP + NSH]
    return full
